# revision 1
# baseline (speedup 1.0000x reference)
"""ANI-style per-species MLP (384->160->128->96->1, CELU) over [B=128, A=512]
atoms with species routing, atom-summed to [B]. 8-core SPMD Trainium2 kernel.

Sharding: atom-parallel. Atoms are grouped by species and dealt round-robin to
the 8 cores so every core sees the same per-species group sizes (padded with
zero-AEV dummy atoms whose contribution is subtracted on the host). Each core
streams its [384, slots*128] transposed AEV block, runs the 4 layers with
per-species weights stationary on the PE (float32r, fp32 PSUM accumulate),
and emits a per-molecule partial sum; the host adds the 8 partials.

CELU is computed as celu(z) = relu(z) + min(alpha*e^(z/alpha) - alpha, 0)
in 3 engine passes (ACT exp; a clamp on DVE 2x or as an ACT relu; a DVE
scalar_tensor_tensor combine); bias constants fold into the exp/relu
per-partition operands and the layer biases fold forward on the host.
"""

import os
import sys

import numpy as np

try:
    import concourse  # noqa: F401
except ImportError:
    sys.path.insert(0, "/opt/trn_rl_repo")

N_CORES = 8
B, A, FEAT = 128, 512, 384
N_SPECIES = 4
H0, H1, H2 = 160, 128, 96
ALPHA = 0.1
LNA = float(np.log(ALPHA))

WPS = 848  # weight-pack columns per species
CPS = 8    # constant-pack columns per species

TRACE = bool(int(os.environ.get("BASSNN_TRACE", "0")))
LAST = {}

_progs = {}


def _maybe_register_ntff_hook():
    try:
        import types

        import antenv
        from antenv import axon_hooks  # noqa: F401
        return
    except ImportError:
        pass
    try:
        import types

        import antenv
        from trn_agent_boot.trn_boot import _ntff_profile_via_ctypes

        mod = types.ModuleType("antenv.axon_hooks")
        holder = [None]
        mod.set_axon_ntff_profile_hook = lambda h: holder.__setitem__(0, h)
        mod.get_axon_ntff_profile_hook = lambda: holder[0]
        sys.modules["antenv.axon_hooks"] = mod
        antenv.axon_hooks = mod
        mod.set_axon_ntff_profile_hook(
            _ntff_profile_via_ctypes("/opt/axon/libaxon_pjrt.so")
        )
    except Exception:
        pass


def _tiles_for_groups(G):
    """Per-species padded group sizes -> list of (species, slot0, n_atoms)."""
    tiles = []
    slot0 = 0
    for s, g in enumerate(G):
        a = 0
        while a < g:
            na = 4 if g - a >= 4 else g - a
            tiles.append((s, slot0 + a, na))
            a += na
        slot0 += g
    return tiles


def _build_program(G, S):
    import concourse.bass as bass
    import concourse.tile as tile
    from concourse import bacc, mybir

    F32 = mybir.dt.float32
    F32R = mybir.dt.float32r
    EXP = mybir.ActivationFunctionType.Exp
    MIN = mybir.AluOpType.min
    MAX = mybir.AluOpType.max
    ADD = mybir.AluOpType.add

    RELU = mybir.ActivationFunctionType.Relu
    SUB = mybir.AluOpType.subtract

    tiles = _tiles_for_groups(G)
    ntiles = len(tiles)
    batches = [list(range(b, min(b + 4, ntiles))) for b in range(0, ntiles, 4)]
    nbatches = len(batches)
    WB0 = WPS * N_SPECIES
    CB0 = CPS * N_SPECIES + 1

    nc = bacc.Bacc("TRN2", target_bir_lowering=False, debug=False,
                   num_devices=N_CORES)
    xt = nc.dram_tensor("xt", [128, 3, S, 128], F32R, kind="ExternalInput").ap()
    wp = nc.dram_tensor("wp", [128, WPS * N_SPECIES + 128 * nbatches], F32R,
                        kind="ExternalInput").ap()
    cp = nc.dram_tensor("cp", [128, CPS * N_SPECIES + 1 + 2 * nbatches], F32,
                        kind="ExternalInput").ap()
    yo = nc.dram_tensor("yo", [1, 128], F32, kind="ExternalOutput").ap()

    with tile.TileContext(nc) as tc:
        with (
            tc.tile_pool(name="wpool", bufs=1) as wpool,
            tc.tile_pool(name="cpool", bufs=1) as cpool,
            tc.tile_pool(name="xpool", bufs=5) as xpool,
            tc.tile_pool(name="epool", bufs=3) as epool,
            tc.tile_pool(name="ypool", bufs=3) as ypool,
            tc.tile_pool(name="y0apool", bufs=6) as y0apool,
            tc.tile_pool(name="opool", bufs=1) as opool,
            tc.tile_pool(name="pp0a", bufs=2, space="PSUM") as pp0a,
            tc.tile_pool(name="pp0b", bufs=2, space="PSUM") as pp0b,
            tc.tile_pool(name="pp1", bufs=2, space="PSUM") as pp1,
            tc.tile_pool(name="pp2", bufs=1, space="PSUM") as pp2,
            tc.tile_pool(name="pp3", bufs=1, space="PSUM") as pp3,
        ):
            w = wpool.tile([128, WPS * N_SPECIES + 128 * nbatches], F32R)
            nc.sync.dma_start(w[:], wp[:])
            c = cpool.tile([128, CPS * N_SPECIES + 1 + 2 * nbatches], F32)
            nc.sync.dma_start(c[:], cp[:])

            def wcol(s, off, n):
                return w[:, s * WPS + off: s * WPS + off + n]

            def ccol(s, k, parts):
                return c[0:parts, s * CPS + k: s * CPS + k + 1]

            p3 = pp3.tile([1, 512], F32)

            # celu(z) - c from psum P (z = P + c), as 3 passes:
            #   E = exp(10P + 10c + ln a)          [ACT]
            #   t' = min(E - a, 0)  in-place       [DVE 2x or ACT relu-form]
            #   y = (P max (-c)) + t'              [DVE stt]
            clamp_flip = [0]

            def celu_unit(y_ap, p_ap, e_tile, ebias, mbias):
                nc.scalar.activation(e_tile, p_ap, EXP, bias=ebias, scale=10.0)
                clamp_flip[0] = (clamp_flip[0] + 1) % 3
                if clamp_flip[0] != 0:
                    nc.vector.tensor_scalar(e_tile, e_tile, ALPHA, 0.0, SUB, MIN)
                    nc.vector.scalar_tensor_tensor(y_ap, p_ap, mbias, e_tile,
                                                   MAX, ADD)
                else:
                    acol = c[0:e_tile.shape[0],
                             CPS * N_SPECIES: CPS * N_SPECIES + 1]
                    nc.scalar.activation(e_tile, e_tile, RELU,
                                         bias=acol, scale=-1.0)
                    nc.vector.scalar_tensor_tensor(y_ap, p_ap, mbias, e_tile,
                                                   MAX, SUB)

            for bi, batch in enumerate(batches):
                y0as = {}
                y0bs = {}
                for j, ti in enumerate(batch):
                    s, a0, na = tiles[ti]
                    N = na * 128
                    xts = xpool.tile([128, 3 * 512], F32R)
                    nc.sync.dma_start(
                        xts[:, 0: 3 * N].rearrange("p (f a m) -> p f a m",
                                                   f=3, a=na, m=128),
                        xt[:, :, a0: a0 + na, :],
                    )

                    p0a = pp0a.tile([128, 512], F32)
                    p0b = pp0b.tile([32, 512], F32)
                    for fc in range(3):
                        rhs = xts[:, fc * N: (fc + 1) * N]
                        nc.tensor.matmul(p0a[:, 0:N], wcol(s, fc * 160, 128),
                                         rhs, start=(fc == 0), stop=(fc == 2))
                        nc.tensor.matmul(p0b[:, 0:N],
                                         wcol(s, fc * 160 + 128, 32),
                                         rhs, start=(fc == 0), stop=(fc == 2))

                    ea = epool.tile([128, 512], F32)
                    y0a = y0apool.tile([128, 512], F32R)
                    celu_unit(y0a[:, 0:N], p0a[:, 0:N], ea[:, 0:N],
                              ccol(s, 0, 128), ccol(s, 2, 128))
                    y0as[ti] = y0a
                    eb = epool.tile([32, 512], F32)
                    y0b = ypool.tile([32, 512], F32R)
                    celu_unit(y0b[:, 0:N], p0b[:, 0:N], eb[:, 0:N],
                              ccol(s, 1, 32), ccol(s, 3, 32))
                    y0bs[ti] = y0b

                for j, ti in enumerate(batch):
                    s, a0, na = tiles[ti]
                    N = na * 128
                    y0a = y0as[ti]
                    y0b = y0bs[ti]
                    p1 = pp1.tile([128, 512], F32)
                    nc.tensor.matmul(p1[:, 0:N], wcol(s, 480, 128),
                                     y0a[:, 0:N], start=True, stop=False)
                    nc.tensor.matmul(p1[:, 0:N], wcol(s, 608, 128)[0:32, :],
                                     y0b[:, 0:N], start=False, stop=True)
                    e1 = epool.tile([128, 512], F32)
                    y1 = ypool.tile([128, 512], F32R)
                    celu_unit(y1[:, 0:N], p1[:, 0:N], e1[:, 0:N],
                              ccol(s, 4, 128), ccol(s, 5, 128))

                    p2 = pp2.tile([96, 512], F32)
                    nc.tensor.matmul(p2[:, 0:N], wcol(s, 736, 96), y1[:, 0:N],
                                     start=True, stop=True)
                    e2 = epool.tile([96, 512], F32)
                    y2 = ypool.tile([96, 512], F32R)
                    celu_unit(y2[:, 0:N], p2[:, 0:N], e2[:, 0:N],
                              ccol(s, 6, 96), ccol(s, 7, 96))

                    nc.tensor.matmul(p3[0:1, 0:N], wcol(s, 832, 1)[0:96, :],
                                     y2[:, 0:N], start=(ti == 0),
                                     stop=(ti == ntiles - 1),
                                     skip_group_check=True)

            t3 = opool.tile([1, 512], F32)
            nc.scalar.copy(t3[:], p3[:])
            f01 = opool.tile([1, 128], F32)
            nc.vector.tensor_add(f01[:], t3[0:1, 0:128], t3[0:1, 128:256])
            f23 = opool.tile([1, 128], F32)
            nc.vector.tensor_add(f23[:], t3[0:1, 256:384], t3[0:1, 384:512])
            fo = opool.tile([1, 128], F32)
            nc.vector.tensor_add(fo[:], f01[:], f23[:])
            nc.sync.dma_start(yo[:], fo[:])

    nc.compile()
    return nc


def _celu64(z):
    return np.where(z > 0, z, ALPHA * np.expm1(z / ALPHA))


def kernel(fullaev, species, W0, b0, W1, b1, W2, b2, W3, b3):
    from concourse import bass_utils

    fullaev = np.ascontiguousarray(np.asarray(fullaev, dtype=np.float32))
    species = np.asarray(species, dtype=np.int32)
    Ws = [np.asarray(w, dtype=np.float32) for w in (W0, W1, W2, W3)]
    bs = [np.asarray(b, dtype=np.float32) for b in (b0, b1, b2, b3)]

    # --- species grouping: per-core slot assignment ---------------------
    ids = [np.where(species == s)[0] for s in range(N_SPECIES)]
    n = [len(i) for i in ids]
    G = []
    for s in range(N_SPECIES):
        g = -(-n[s] // N_CORES) if n[s] else 0
        g += g % 2
        G.append(g)
    S = sum(G)
    key = tuple(G)
    if key not in _progs:
        _progs[key] = _build_program(G, S)
    nc = _progs[key]

    # --- fold constants (float64) ---------------------------------------
    corr = np.zeros(N_SPECIES)
    Kdum = np.zeros(N_SPECIES)
    tiles = _tiles_for_groups(G)
    batches = [list(range(b, min(b + 4, len(tiles)))) for b in range(0, len(tiles), 4)]
    nbatches = len(batches)
    CB0 = CPS * N_SPECIES + 1
    WB0 = WPS * N_SPECIES
    cpack = np.zeros((128, CB0 + 2 * nbatches), np.float32)
    cpack[:, CB0 - 1] = ALPHA
    wpack = np.zeros((128, WB0 + 128 * nbatches), np.float32)
    for bi, batch in enumerate(batches):
        for j, ti in enumerate(batch):
            s = tiles[ti][0]
            b0s = bs[0][s].astype(np.float64)
            cpack[32 * j: 32 * j + 32, CB0 + 2 * bi] = (
                10.0 * b0s[128:] + LNA)
            cpack[32 * j: 32 * j + 32, CB0 + 2 * bi + 1] = -b0s[128:]
            wpack[32 * j: 32 * j + 32, WB0 + 128 * bi: WB0 + 128 * (bi + 1)] = (
                Ws[1][s][:, 128:].T)
    for s in range(N_SPECIES):
        w0, w1, w2, w3 = (w[s].astype(np.float64) for w in Ws)
        bb0, bb1, bb2, bb3 = (b[s].astype(np.float64) for b in bs)
        c1 = bb1 + w1 @ bb0
        c2 = bb2 + w2 @ c1
        corr[s] = bb3[0] + w3[0] @ c2
        y = _celu64(bb0)
        y = _celu64(w1 @ y + bb1)
        y = _celu64(w2 @ y + bb2)
        Kdum[s] = w3[0] @ y + bb3[0]

        cb = s * CPS
        cpack[:, cb + 0] = 10.0 * bb0[:128] + LNA
        cpack[:32, cb + 1] = 10.0 * bb0[128:] + LNA
        cpack[:, cb + 2] = -bb0[:128]
        cpack[:32, cb + 3] = -bb0[128:]
        cpack[:, cb + 4] = 10.0 * c1 + LNA
        cpack[:, cb + 5] = -c1
        cpack[:96, cb + 6] = 10.0 * c2 + LNA
        cpack[:96, cb + 7] = -c2

        wb = s * WPS
        for fc in range(3):
            blk = Ws[0][s][:, fc * 128:(fc + 1) * 128].T  # [128in, 160out]
            wpack[:, wb + fc * 160: wb + fc * 160 + 128] = blk[:, :128]
            wpack[:, wb + fc * 160 + 128: wb + (fc + 1) * 160] = blk[:, 128:]
        wpack[:, wb + 480: wb + 608] = Ws[1][s][:, :128].T
        wpack[:32, wb + 608: wb + 736] = Ws[1][s][:, 128:].T
        wpack[:, wb + 736: wb + 832] = Ws[2][s].T
        wpack[:96, wb + 832] = Ws[3][s][0, :]

    # --- per-core transposed, species-sorted AEV blocks -----------------
    in_maps = []
    dummy_counts = np.zeros((N_CORES, N_SPECIES), np.int64)
    for cid in range(N_CORES):
        xtc = np.zeros((128, 3, S, 128), np.float32)
        slot0 = 0
        for s in range(N_SPECIES):
            mine = ids[s][cid::N_CORES]
            nr = len(mine)
            dummy_counts[cid, s] = G[s] - nr
            if nr:
                g = fullaev[:, mine, :]               # [128, nr, 384]
                t = g.transpose(2, 1, 0)              # [384, nr, 128]
                xtc[:, :, slot0: slot0 + nr, :] = (
                    t.reshape(3, 128, nr, 128).transpose(1, 0, 2, 3)
                )
            slot0 += G[s]
        in_maps.append({"xt": xtc, "wp": wpack, "cp": cpack})

    if TRACE:
        _maybe_register_ntff_hook()
    res = bass_utils.run_bass_kernel_spmd(
        nc, in_maps, core_ids=list(range(N_CORES)), trace=TRACE
    )
    LAST["exec_time_ns"] = res.exec_time_ns
    LAST["trace"] = res.instructions_and_trace[1] if res.instructions_and_trace else None

    total_corr = 0.0
    for s in range(N_SPECIES):
        total_corr += N_CORES * G[s] * corr[s] - dummy_counts[:, s].sum() * Kdum[s]
    out = np.zeros(128, np.float64)
    for cid in range(N_CORES):
        out += res.results[cid]["yo"][0].astype(np.float64)
    out += total_corr
    return out.astype(np.float32)



# revision 10
# speedup vs baseline: 1.0088x; 1.0088x over previous
"""ANI-style per-species MLP (384->160->128->96->1, CELU) over [B=128, A=512]
atoms with species routing, atom-summed to [B]. 8-core SPMD Trainium2 kernel.

v2: atom-parallel sharding as before (species-grouped atoms dealt round-robin
to 8 cores, zero-AEV dummy padding corrected on host), with:
  - AEV streamed as fp8e4m3 (rhs) against bf16 weights (lhsT): 4x less DMA.
  - bf16 activations in SBUF; f32 PSUM accumulate.
  - celu split: V1 = exp(ACT) + clamp(DVE ts, bf16 4x mode) + stt(DVE);
    V2 (layer1) = exp+relu on ACT + clamp + tensor_add(DVE 2x) to balance
    ACT vs DVE occupancy.
  - L0's 32-wide output chunk for the 4 tiles of a batch lands in ONE psum
    bank at partition offsets 32j -> one merged celu instead of 4.
  - software-pipelined emission B(k) -> A(k+1) -> C(k) so the PE queue never
    waits on celu latency; PSUM pools shared (4 + 3 + 1 banks).
"""

import os
import sys

import numpy as np

try:
    import concourse  # noqa: F401
except ImportError:
    sys.path.insert(0, "/opt/trn_rl_repo")

N_CORES = 8
B, A, FEAT = 128, 512, 384
N_SPECIES = 4
H0, H1, H2 = 160, 128, 96
ALPHA = 0.1
LNA = float(np.log(ALPHA))

WPS = 3 * 160 + 128 + 96 + 1  # 705 weight-pack columns per species
CPS = 6                       # constant-pack columns per species

X_FP8 = bool(int(os.environ.get("BASSNN_X_FP8", "1")))

TRACE = bool(int(os.environ.get("BASSNN_TRACE", "0")))
LAST = {}

_progs = {}


def _maybe_register_ntff_hook():
    try:
        import types

        import antenv
        from antenv import axon_hooks  # noqa: F401
        return
    except ImportError:
        pass
    try:
        import types

        import antenv
        from trn_agent_boot.trn_boot import _ntff_profile_via_ctypes

        mod = types.ModuleType("antenv.axon_hooks")
        holder = [None]
        mod.set_axon_ntff_profile_hook = lambda h: holder.__setitem__(0, h)
        mod.get_axon_ntff_profile_hook = lambda: holder[0]
        sys.modules["antenv.axon_hooks"] = mod
        antenv.axon_hooks = mod
        mod.set_axon_ntff_profile_hook(
            _ntff_profile_via_ctypes("/opt/axon/libaxon_pjrt.so")
        )
    except Exception:
        pass


def _tiles_for_groups(G):
    """Per-species padded group sizes -> list of (species, slot0, n_atoms)."""
    tiles = []
    slot0 = 0
    for s, g in enumerate(G):
        a = 0
        while a < g:
            na = 4 if g - a >= 4 else g - a
            tiles.append((s, slot0 + a, na))
            a += na
        slot0 += g
    return tiles


def _batches_for_tiles(tiles):
    return [list(range(b, min(b + 4, len(tiles)))) for b in range(0, len(tiles), 4)]


def _build_program(G, S):
    import concourse.bass as bass  # noqa: F401
    import concourse.tile as tile
    from concourse import bacc, mybir

    F32 = mybir.dt.float32
    BF16 = mybir.dt.bfloat16
    X8 = mybir.dt.float8e4 if X_FP8 else BF16
    EXP = mybir.ActivationFunctionType.Exp
    RELU = mybir.ActivationFunctionType.Relu
    MIN = mybir.AluOpType.min
    MAX = mybir.AluOpType.max
    ADD = mybir.AluOpType.add
    SUB = mybir.AluOpType.subtract

    tiles = _tiles_for_groups(G)
    ntiles = len(tiles)
    batches = _batches_for_tiles(tiles)
    nbatches = len(batches)
    WB0 = WPS * N_SPECIES
    CB0 = CPS * N_SPECIES
    WCOLS = WB0 + 128 * nbatches
    CCOLS = CB0 + 2 * nbatches
    # batch k covers consecutive slots [bslot0[k], bslot0[k] + bna[k])
    bslot0 = [tiles[bt[0]][1] for bt in batches]
    bna = [sum(tiles[ti][2] for ti in bt) for bt in batches]
    XB = max(bna) * 384

    nc = bacc.Bacc("TRN2", target_bir_lowering=False, debug=False,
                   num_devices=N_CORES)
    xt = nc.dram_tensor("xt", [128, S * 384], X8, kind="ExternalInput").ap()
    wp = nc.dram_tensor("wp", [128, WCOLS], BF16, kind="ExternalInput").ap()
    cp = nc.dram_tensor("cp", [128, CCOLS], F32, kind="ExternalInput").ap()
    yo = nc.dram_tensor("yo", [1, 128], F32, kind="ExternalOutput").ap()

    with tile.TileContext(nc) as tc:
        with (
            tc.tile_pool(name="wpool", bufs=1) as wpool,
            tc.tile_pool(name="cpool", bufs=1) as cpool,
            tc.tile_pool(name="xpool", bufs=3) as xpool,
            tc.tile_pool(name="epool", bufs=6) as epool,
            tc.tile_pool(name="rpool", bufs=3) as rpool,
            tc.tile_pool(name="y0apool", bufs=6) as y0apool,
            tc.tile_pool(name="y0bpool", bufs=2) as y0bpool,
            tc.tile_pool(name="y1pool", bufs=5) as y1pool,
            tc.tile_pool(name="y2pool", bufs=6) as y2pool,
            tc.tile_pool(name="opool", bufs=1) as opool,
            tc.tile_pool(name="ppA", bufs=4, space="PSUM") as ppA,
            tc.tile_pool(name="ppB", bufs=3, space="PSUM") as ppB,
            tc.tile_pool(name="pp3", bufs=1, space="PSUM") as pp3,
        ):
            w = wpool.tile([128, WCOLS], BF16)
            nc.sync.dma_start(w[:], wp[:])
            c = cpool.tile([128, CCOLS], F32)
            nc.sync.dma_start(c[:], cp[:])

            def wcol(s, off, n):
                return w[:, s * WPS + off: s * WPS + off + n]

            def ccol(s, k, parts):
                return c[0:parts, s * CPS + k: s * CPS + k + 1]

            p3 = pp3.tile([1, 512], F32)

            def celu_v1(y_ap, p_ap, e_ap, ebias, mbias):
                # y = (P max mbias) + min(alpha*e^(10P+ebias) - alpha, 0)
                nc.scalar.activation(e_ap, p_ap, EXP, bias=ebias, scale=10.0)
                nc.vector.tensor_scalar(e_ap, e_ap, ALPHA, 0.0, SUB, MIN)
                nc.vector.scalar_tensor_tensor(y_ap, p_ap, mbias, e_ap,
                                               MAX, ADD)

            def celu_v2(y_ap, p_ap, e_ap, r_ap, ebias, rbias):
                # y = relu(P + rbias) + min(alpha*e^(10P+ebias) - alpha, 0)
                nc.scalar.activation(e_ap, p_ap, EXP, bias=ebias, scale=10.0)
                nc.scalar.activation(r_ap, p_ap, RELU, bias=rbias, scale=1.0)
                nc.vector.tensor_scalar(e_ap, e_ap, ALPHA, 0.0, SUB, MIN)
                nc.vector.tensor_add(y_ap, r_ap, e_ap)

            xts = {}

            def dma_batch(k):
                t = xpool.tile([128, XB], X8)
                nc.sync.dma_start(
                    t[:, 0: bna[k] * 384],
                    xt[:, bslot0[k] * 384: (bslot0[k] + bna[k]) * 384],
                )
                xts[k] = t

            y0as = {}
            y0bs = {}
            y1s = {}
            y2s = {}

            def emitA(k):
                batch = batches[k]
                L = len(batch)
                xv = xts[k].rearrange("p (a f m) -> p a f m",
                                      a=XB // 384, f=3, m=128)
                p0b = ppB.tile([128, 512], F32, tag="pB")
                p0as = {}
                for j, ti in enumerate(batch):
                    s, a0, na = tiles[ti]
                    N = na * 128
                    aoff = a0 - bslot0[k]
                    p0a = ppA.tile([128, 512], F32, tag="pA")
                    for fc in range(3):
                        rhs = xv[:, aoff: aoff + na, fc, :]
                        nc.tensor.matmul(p0a[:, 0:N],
                                         wcol(s, fc * 160, 128),
                                         rhs, start=(fc == 0), stop=(fc == 2))
                        nc.tensor.matmul(p0b[32 * j: 32 * j + 32, 0:N],
                                         wcol(s, fc * 160 + 128, 32),
                                         rhs, start=(fc == 0), stop=(fc == 2),
                                         tile_position=(0, 32 * j))
                    p0as[ti] = p0a
                    ea = epool.tile([128, 512], BF16)
                    y0a = y0apool.tile([128, 512], BF16)
                    celu_v1(y0a[:, 0:N], p0a[:, 0:N], ea[:, 0:N],
                            ccol(s, 0, 128), ccol(s, 1, 128))
                    y0as[ti] = y0a
                # merged L0b celu across the whole batch
                em = epool.tile([128, 512], BF16)
                y0b = y0bpool.tile([128, 512], BF16)
                P = 32 * L
                celu_v1(y0b[0:P, :], p0b[0:P, :], em[0:P, :],
                        c[0:P, CB0 + 2 * k: CB0 + 2 * k + 1],
                        c[0:P, CB0 + 2 * k + 1: CB0 + 2 * k + 2])
                y0bs[k] = y0b

            def emitB(k):
                batch = batches[k]
                y0b = y0bs[k]
                p1s = {}
                for j, ti in enumerate(batch):
                    s, a0, na = tiles[ti]
                    N = na * 128
                    p1 = ppA.tile([128, 512], F32, tag="pA")
                    nc.tensor.matmul(p1[:, 0:N], wcol(s, 480, 128),
                                     y0as[ti][:, 0:N], start=True, stop=False)
                    nc.tensor.matmul(
                        p1[:, 0:N],
                        w[32 * j: 32 * j + 32,
                          WB0 + 128 * k: WB0 + 128 * (k + 1)],
                        y0b[32 * j: 32 * j + 32, 0:N],
                        start=False, stop=True,
                        tile_position=(32 * j, 0))
                    p1s[ti] = p1
                    e1 = epool.tile([128, 512], BF16)
                    r1 = rpool.tile([128, 512], BF16)
                    y1 = y1pool.tile([128, 512], BF16)
                    celu_v2(y1[:, 0:N], p1[:, 0:N], e1[:, 0:N], r1[:, 0:N],
                            ccol(s, 2, 128), ccol(s, 3, 128))
                    y1s[ti] = y1
                for j, ti in enumerate(batch):
                    s, a0, na = tiles[ti]
                    N = na * 128
                    p2 = ppB.tile([96, 512], F32, tag="pB")
                    nc.tensor.matmul(p2[:, 0:N], wcol(s, 608, 96),
                                     y1s[ti][:, 0:N], start=True, stop=True)
                    e2 = epool.tile([96, 512], BF16)
                    y2 = y2pool.tile([96, 512], BF16)
                    celu_v1(y2[:, 0:N], p2[:, 0:N], e2[:, 0:N],
                            ccol(s, 4, 96), ccol(s, 5, 96))
                    y2s[ti] = y2

            def emitC(k):
                batch = batches[k]
                for j, ti in enumerate(batch):
                    s, a0, na = tiles[ti]
                    N = na * 128
                    nc.tensor.matmul(p3[0:1, 0:N],
                                     wcol(s, 704, 1)[0:96, :],
                                     y2s[ti][0:96, 0:N],
                                     start=(ti == 0), stop=(ti == ntiles - 1),
                                     skip_group_check=True)

            dma_batch(0)
            emitA(0)
            for k in range(nbatches):
                if k + 1 < nbatches:
                    dma_batch(k + 1)
                emitB(k)
                if k + 1 < nbatches:
                    emitA(k + 1)
                emitC(k)

            t3 = opool.tile([1, 512], F32)
            nc.scalar.copy(t3[:], p3[:])
            f01 = opool.tile([1, 128], F32)
            nc.vector.tensor_add(f01[:], t3[0:1, 0:128], t3[0:1, 128:256])
            f23 = opool.tile([1, 128], F32)
            nc.vector.tensor_add(f23[:], t3[0:1, 256:384], t3[0:1, 384:512])
            fo = opool.tile([1, 128], F32)
            nc.vector.tensor_add(fo[:], f01[:], f23[:])
            nc.sync.dma_start(yo[:], fo[:])

    nc.compile()
    return nc


def _celu64(z):
    return np.where(z > 0, z, ALPHA * np.expm1(np.minimum(z, 0) / ALPHA))


def _bf16_round(x):
    import ml_dtypes
    return np.asarray(x, np.float32).astype(ml_dtypes.bfloat16).astype(np.float64)


def kernel(fullaev, species, W0, b0, W1, b1, W2, b2, W3, b3):
    import ml_dtypes
    from concourse import bass_utils, mybir

    fullaev = np.ascontiguousarray(np.asarray(fullaev, dtype=np.float32))
    species = np.asarray(species, dtype=np.int32)
    Ws = [np.asarray(w, dtype=np.float32) for w in (W0, W1, W2, W3)]
    bs = [np.asarray(b, dtype=np.float32) for b in (b0, b1, b2, b3)]

    # --- species grouping: per-core slot assignment ---------------------
    ids = [np.where(species == s)[0] for s in range(N_SPECIES)]
    n = [len(i) for i in ids]
    G = [-(-n[s] // N_CORES) if n[s] else 0 for s in range(N_SPECIES)]
    S = sum(G)
    key = (tuple(G), X_FP8)
    if key not in _progs:
        _progs[key] = _build_program(G, S)
    nc = _progs[key]

    tiles = _tiles_for_groups(G)
    batches = _batches_for_tiles(tiles)
    nbatches = len(batches)
    WB0 = WPS * N_SPECIES
    CB0 = CPS * N_SPECIES
    WCOLS = WB0 + 128 * nbatches
    CCOLS = CB0 + 2 * nbatches

    # --- fold constants (float64, with bf16-rounded weights) ------------
    cpack = np.zeros((128, CCOLS), np.float32)
    wpack = np.zeros((128, WCOLS), np.float32)
    c3 = np.zeros(N_SPECIES)
    K0 = np.zeros(N_SPECIES)
    c1s = {}
    for s in range(N_SPECIES):
        w1, w2, w3 = (_bf16_round(Ws[l][s]) for l in (1, 2, 3))
        bb0, bb1, bb2, bb3 = (b[s].astype(np.float64) for b in bs)
        c1 = bb1 + w1 @ bb0
        c1s[s] = c1
        c3[s] = bb3[0] + w3[0] @ bb2
        # device contribution of a dummy (zero-AEV) atom, excluding c3
        y0d = _celu64(bb0) - bb0
        y1d = _celu64(w1 @ y0d + c1)
        y2d = _celu64(w2 @ y1d + bb2) - bb2
        K0[s] = w3[0] @ y2d

        cb = s * CPS
        cpack[:, cb + 0] = 10.0 * bb0[:128] + LNA
        cpack[:, cb + 1] = -bb0[:128]
        cpack[:, cb + 2] = 10.0 * c1 + LNA
        cpack[:, cb + 3] = c1
        cpack[:96, cb + 4] = 10.0 * bb2 + LNA
        cpack[:96, cb + 5] = -bb2

        wb = s * WPS
        for fc in range(3):
            blk = Ws[0][s][:, fc * 128:(fc + 1) * 128].T  # [128in, 160out]
            wpack[:, wb + fc * 160: wb + fc * 160 + 160] = blk
        wpack[:, wb + 480: wb + 608] = Ws[1][s][:, :128].T
        wpack[:, wb + 608: wb + 704] = Ws[2][s].T
        wpack[:96, wb + 704] = Ws[3][s][0, :]

    for bi, batch in enumerate(batches):
        for j, ti in enumerate(batch):
            s = tiles[ti][0]
            b0b = bs[0][s].astype(np.float64)[128:]
            cpack[32 * j: 32 * j + 32, CB0 + 2 * bi] = 10.0 * b0b + LNA
            cpack[32 * j: 32 * j + 32, CB0 + 2 * bi + 1] = -b0b
            wpack[32 * j: 32 * j + 32,
                  WB0 + 128 * bi: WB0 + 128 * (bi + 1)] = Ws[1][s][:, 128:].T

    wpack_b = wpack.astype(ml_dtypes.bfloat16)
    x_np_dtype = mybir.dt.np(mybir.dt.float8e4 if X_FP8 else mybir.dt.bfloat16)

    # --- per-core transposed, species-sorted AEV blocks -----------------
    in_maps = []
    dummy_counts = np.zeros((N_CORES, N_SPECIES), np.int64)
    for cid in range(N_CORES):
        xtc = np.zeros((128, S, 3, 128), np.float32)
        slot0 = 0
        for s in range(N_SPECIES):
            mine = ids[s][cid::N_CORES]
            nr = len(mine)
            dummy_counts[cid, s] = G[s] - nr
            if nr:
                g = fullaev[:, mine, :]               # [128, nr, 384]
                t = g.transpose(2, 1, 0)              # [384, nr, 128]
                xtc[:, slot0: slot0 + nr, :, :] = (
                    t.reshape(3, 128, nr, 128).transpose(1, 2, 0, 3)
                )
            slot0 += G[s]
        xq = xtc.reshape(128, S * 384).astype(x_np_dtype)
        in_maps.append({"xt": xq, "wp": wpack_b, "cp": cpack})

    if TRACE:
        _maybe_register_ntff_hook()
    res = bass_utils.run_bass_kernel_spmd(
        nc, in_maps, core_ids=list(range(N_CORES)), trace=TRACE
    )
    LAST["exec_time_ns"] = res.exec_time_ns
    LAST["trace"] = res.instructions_and_trace[1] if res.instructions_and_trace else None

    out = np.zeros(128, np.float64)
    for cid in range(N_CORES):
        out += res.results[cid]["yo"][0].astype(np.float64)
    for s in range(N_SPECIES):
        out += n[s] * c3[s] - dummy_counts[:, s].sum() * K0[s]
    return out.astype(np.float32)


# revision 12
# speedup vs baseline: 1.1359x; 1.1260x over previous
"""ANI-style per-species MLP (384->160->128->96->1, CELU) over [B=128, A=512]
atoms with species routing, atom-summed to [B]. 8-core SPMD Trainium2 kernel.

v2: atom-parallel sharding as before (species-grouped atoms dealt round-robin
to 8 cores, zero-AEV dummy padding corrected on host), with:
  - AEV streamed as fp8e4m3 (rhs) against bf16 weights (lhsT): 4x less DMA.
  - bf16 activations in SBUF; f32 PSUM accumulate.
  - celu split: V1 = exp(ACT) + clamp(DVE ts, bf16 4x mode) + stt(DVE);
    V2 (layer1) = exp+relu on ACT + clamp + tensor_add(DVE 2x) to balance
    ACT vs DVE occupancy.
  - L0's 32-wide output chunk for the 4 tiles of a batch lands in ONE psum
    bank at partition offsets 32j -> one merged celu instead of 4.
  - software-pipelined emission B(k) -> A(k+1) -> C(k) so the PE queue never
    waits on celu latency; PSUM pools shared (4 + 3 + 1 banks).
"""

import os
import sys

import numpy as np

try:
    import concourse  # noqa: F401
except ImportError:
    sys.path.insert(0, "/opt/trn_rl_repo")

N_CORES = 8
B, A, FEAT = 128, 512, 384
N_SPECIES = 4
H0, H1, H2 = 160, 128, 96
ALPHA = 0.1
LNA = float(np.log(ALPHA))

WPS = 3 * 160 + 128 + 96 + 1  # 705 weight-pack columns per species
CPS = 6                       # constant-pack columns per species

X_FP8 = bool(int(os.environ.get("BASSNN_X_FP8", "1")))

TRACE = bool(int(os.environ.get("BASSNN_TRACE", "0")))
LAST = {}

_progs = {}


def _maybe_register_ntff_hook():
    try:
        import types

        import antenv
        from antenv import axon_hooks  # noqa: F401
        return
    except ImportError:
        pass
    try:
        import types

        import antenv
        from trn_agent_boot.trn_boot import _ntff_profile_via_ctypes

        mod = types.ModuleType("antenv.axon_hooks")
        holder = [None]
        mod.set_axon_ntff_profile_hook = lambda h: holder.__setitem__(0, h)
        mod.get_axon_ntff_profile_hook = lambda: holder[0]
        sys.modules["antenv.axon_hooks"] = mod
        antenv.axon_hooks = mod
        mod.set_axon_ntff_profile_hook(
            _ntff_profile_via_ctypes("/opt/axon/libaxon_pjrt.so")
        )
    except Exception:
        pass


def _tiles_for_groups(G):
    """Per-species padded group sizes -> list of (species, slot0, n_atoms)."""
    tiles = []
    slot0 = 0
    for s, g in enumerate(G):
        a = 0
        while a < g:
            na = 4 if g - a >= 4 else g - a
            tiles.append((s, slot0 + a, na))
            a += na
        slot0 += g
    return tiles


def _batches_for_tiles(tiles):
    return [list(range(b, min(b + 4, len(tiles)))) for b in range(0, len(tiles), 4)]


def _build_program(G, S):
    import concourse.bass as bass  # noqa: F401
    import concourse.tile as tile
    from concourse import bacc, mybir

    F32 = mybir.dt.float32
    BF16 = mybir.dt.bfloat16
    X8 = mybir.dt.float8e4 if X_FP8 else BF16
    EXP = mybir.ActivationFunctionType.Exp
    RELU = mybir.ActivationFunctionType.Relu
    MIN = mybir.AluOpType.min
    MAX = mybir.AluOpType.max
    ADD = mybir.AluOpType.add
    SUB = mybir.AluOpType.subtract

    tiles = _tiles_for_groups(G)
    ntiles = len(tiles)
    batches = _batches_for_tiles(tiles)
    nbatches = len(batches)
    WB0 = WPS * N_SPECIES
    CB0 = CPS * N_SPECIES
    WCOLS = WB0 + 128 * nbatches
    CCOLS = CB0 + 2 * nbatches
    # batch k covers consecutive slots [bslot0[k], bslot0[k] + bna[k])
    bslot0 = [tiles[bt[0]][1] for bt in batches]
    bna = [sum(tiles[ti][2] for ti in bt) for bt in batches]
    XB = max(bna) * 384

    nc = bacc.Bacc("TRN2", target_bir_lowering=False, debug=False,
                   num_devices=N_CORES)
    xt = nc.dram_tensor("xt", [128, S * 384], X8, kind="ExternalInput").ap()
    wp = nc.dram_tensor("wp", [128, WCOLS], BF16, kind="ExternalInput").ap()
    cp = nc.dram_tensor("cp", [128, CCOLS], F32, kind="ExternalInput").ap()
    yo = nc.dram_tensor("yo", [1, 128], F32, kind="ExternalOutput").ap()

    with tile.TileContext(nc) as tc:
        with (
            tc.tile_pool(name="wpool", bufs=1) as wpool,
            tc.tile_pool(name="cpool", bufs=1) as cpool,
            tc.tile_pool(name="xpool", bufs=3) as xpool,
            tc.tile_pool(name="epool", bufs=6) as epool,
            tc.tile_pool(name="tpool", bufs=6) as tpool,
            tc.tile_pool(name="rpool", bufs=3) as rpool,
            tc.tile_pool(name="y0apool", bufs=6) as y0apool,
            tc.tile_pool(name="y0bpool", bufs=2) as y0bpool,
            tc.tile_pool(name="y1pool", bufs=5) as y1pool,
            tc.tile_pool(name="y2pool", bufs=6) as y2pool,
            tc.tile_pool(name="opool", bufs=1) as opool,
            tc.tile_pool(name="ppa", bufs=2, space="PSUM") as ppa,
            tc.tile_pool(name="pp1", bufs=2, space="PSUM") as pp1,
            tc.tile_pool(name="ppb0", bufs=1, space="PSUM") as ppb0,
            tc.tile_pool(name="ppb2", bufs=2, space="PSUM") as ppb2,
            tc.tile_pool(name="pp3", bufs=1, space="PSUM") as pp3,
        ):
            # weights/constants on the gpsimd DMA queue, x batches on sync:
            # descriptor generation and transfers overlap.
            w = wpool.tile([128, WCOLS], BF16)
            nc.gpsimd.dma_start(w[:], wp[:])
            c = cpool.tile([128, CCOLS], F32)
            nc.gpsimd.dma_start(c[:], cp[:])

            def wcol(s, off, n):
                return w[:, s * WPS + off: s * WPS + off + n]

            def ccol(s, k, parts):
                return c[0:parts, s * CPS + k: s * CPS + k + 1]

            p3 = pp3.tile([1, 512], F32)

            def celu_v1(y_ap, p_ap, e_ap, t_ap, ebias, mbias):
                # y = (P max mbias) + min(alpha*e^(10P+ebias) - alpha, 0)
                nc.scalar.activation(e_ap, p_ap, EXP, bias=ebias, scale=10.0)
                nc.vector.tensor_scalar(t_ap, e_ap, ALPHA, 0.0, SUB, MIN)
                nc.vector.scalar_tensor_tensor(y_ap, p_ap, mbias, t_ap,
                                               MAX, ADD)

            def celu_v2(y_ap, p_ap, e_ap, t_ap, r_ap, ebias, rbias):
                # y = relu(P + rbias) + min(alpha*e^(10P+ebias) - alpha, 0)
                nc.scalar.activation(e_ap, p_ap, EXP, bias=ebias, scale=10.0)
                nc.scalar.activation(r_ap, p_ap, RELU, bias=rbias, scale=1.0)
                nc.vector.tensor_scalar(t_ap, e_ap, ALPHA, 0.0, SUB, MIN)
                nc.vector.tensor_add(y_ap, r_ap, t_ap)

            xts = {}

            def dma_batch(k):
                t = xpool.tile([128, XB], X8, name="xts")
                nc.sync.dma_start(
                    t[:, 0: bna[k] * 384],
                    xt[:, bslot0[k] * 384: (bslot0[k] + bna[k]) * 384],
                )
                xts[k] = t

            y0as = {}
            y0bs = {}
            y1s = {}
            y2s = {}
            p0bs = {}

            def l0_mm_thunks(k):
                """Batch k's L0 matmuls as 12 thunks (2 mms each)."""
                batch = batches[k]
                xv = xts[k].rearrange("p (a f m) -> p a f m",
                                      a=XB // 384, f=3, m=128)
                p0b = ppb0.tile([128, 512], F32, tag="pb", name="p0b")
                p0bs[k] = p0b
                p0as = {}
                thunks = []
                for j, ti in enumerate(batch):
                    s, a0, na = tiles[ti]
                    N = na * 128
                    aoff = a0 - bslot0[k]
                    p0a = ppa.tile([128, 512], F32, tag="pa", name="p0a")
                    p0as[ti] = p0a

                    def mk(j=j, s=s, na=na, N=N, aoff=aoff, p0a=p0a, fc=0):
                        rhs = xv[:, aoff: aoff + na, fc, :]
                        nc.tensor.matmul(p0a[:, 0:N],
                                         wcol(s, fc * 160, 128),
                                         rhs, start=(fc == 0), stop=(fc == 2))
                        nc.tensor.matmul(p0b[32 * j: 32 * j + 32, 0:N],
                                         wcol(s, fc * 160 + 128, 32),
                                         rhs, start=(fc == 0), stop=(fc == 2),
                                         tile_position=(0, 32 * j))
                    for fc in range(3):
                        thunks.append(
                            lambda mk=mk, fc=fc: mk(fc=fc))
                return thunks, p0as

            def l0a_celu(k, j, p0as):
                ti = batches[k][j]
                s, a0, na = tiles[ti]
                N = na * 128
                ea = epool.tile([128, 512], BF16, name="ea")
                ta = tpool.tile([128, 512], BF16, name="ta")
                y0a = y0apool.tile([128, 512], BF16, name="y0a")
                celu_v1(y0a[:, 0:N], p0as[ti][:, 0:N], ea[:, 0:N],
                        ta[:, 0:N], ccol(s, 0, 128), ccol(s, 1, 128))
                y0as[ti] = y0a

            def l0b_celu(k):
                L = len(batches[k])
                em = epool.tile([128, 512], BF16, name="em")
                tm = tpool.tile([128, 512], BF16, name="tm")
                y0b = y0bpool.tile([128, 512], BF16, name="y0b")
                P = 32 * L
                celu_v1(y0b[0:P, :], p0bs[k][0:P, :], em[0:P, :], tm[0:P, :],
                        c[0:P, CB0 + 2 * k: CB0 + 2 * k + 1],
                        c[0:P, CB0 + 2 * k + 1: CB0 + 2 * k + 2])
                y0bs[k] = y0b

            def steady(k):
                """B(k) with A(k+1)'s L0 matmuls interleaved, then C(k)."""
                batch = batches[k]
                y0b = y0bs[k]
                if k + 2 < nbatches:
                    dma_batch(k + 2)
                p1s = {}
                for j, ti in enumerate(batch):
                    s, a0, na = tiles[ti]
                    N = na * 128
                    p1 = pp1.tile([128, 512], F32, tag="p1", name="p1")
                    nc.tensor.matmul(p1[:, 0:N], wcol(s, 480, 128),
                                     y0as[ti][:, 0:N], start=True, stop=False)
                    nc.tensor.matmul(
                        p1[:, 0:N],
                        w[32 * j: 32 * j + 32,
                          WB0 + 128 * k: WB0 + 128 * (k + 1)],
                        y0b[32 * j: 32 * j + 32, 0:N],
                        start=False, stop=True,
                        tile_position=(32 * j, 0))
                    p1s[ti] = p1
                for j, ti in enumerate(batch):
                    s, a0, na = tiles[ti]
                    N = na * 128
                    e1 = epool.tile([128, 512], BF16, name="e1")
                    t1 = tpool.tile([128, 512], BF16, name="t1")
                    r1 = rpool.tile([128, 512], BF16, name="r1")
                    y1 = y1pool.tile([128, 512], BF16, name="y1")
                    celu_v2(y1[:, 0:N], p1s[ti][:, 0:N], e1[:, 0:N],
                            t1[:, 0:N], r1[:, 0:N],
                            ccol(s, 2, 128), ccol(s, 3, 128))
                    y1s[ti] = y1
                if k + 1 < nbatches:
                    thunks, p0as_next = l0_mm_thunks(k + 1)
                else:
                    thunks, p0as_next = [], None
                chunks = [thunks[3 * i: 3 * i + 3] for i in range(4)]
                for j, ti in enumerate(batch):
                    for th in chunks[j]:
                        th()
                    s, a0, na = tiles[ti]
                    N = na * 128
                    p2 = ppb2.tile([96, 512], F32, tag="p2", name="p2")
                    nc.tensor.matmul(p2[:, 0:N], wcol(s, 608, 96),
                                     y1s[ti][:, 0:N], start=True, stop=True)
                    e2 = epool.tile([96, 512], BF16, name="e2")
                    t2 = tpool.tile([96, 512], BF16, name="t2")
                    y2 = y2pool.tile([96, 512], BF16, name="y2")
                    celu_v1(y2[:, 0:N], p2[:, 0:N], e2[:, 0:N], t2[:, 0:N],
                            ccol(s, 4, 96), ccol(s, 5, 96))
                    y2s[ti] = y2
                    if p0as_next is not None and j < len(batches[k + 1]):
                        l0a_celu(k + 1, j, p0as_next)
                for ch in chunks[len(batch):]:
                    for th in ch:
                        th()
                for j, ti in enumerate(batch):
                    s, a0, na = tiles[ti]
                    N = na * 128
                    nc.tensor.matmul(p3[0:1, 0:N],
                                     wcol(s, 704, 1)[0:96, :],
                                     y2s[ti][0:96, 0:N],
                                     start=(ti == 0), stop=(ti == ntiles - 1),
                                     skip_group_check=True)
                if p0as_next is not None:
                    for j in range(len(batch), len(batches[k + 1])):
                        l0a_celu(k + 1, j, p0as_next)
                    l0b_celu(k + 1)

            dma_batch(0)
            if nbatches > 1:
                dma_batch(1)
            thunks0, p0as0 = l0_mm_thunks(0)
            for th in thunks0:
                th()
            for j in range(len(batches[0])):
                l0a_celu(0, j, p0as0)
            l0b_celu(0)
            for k in range(nbatches):
                steady(k)

            t3 = opool.tile([1, 512], F32)
            nc.scalar.copy(t3[:], p3[:])
            f01 = opool.tile([1, 128], F32)
            nc.vector.tensor_add(f01[:], t3[0:1, 0:128], t3[0:1, 128:256])
            f23 = opool.tile([1, 128], F32)
            nc.vector.tensor_add(f23[:], t3[0:1, 256:384], t3[0:1, 384:512])
            fo = opool.tile([1, 128], F32)
            nc.vector.tensor_add(fo[:], f01[:], f23[:])
            nc.sync.dma_start(yo[:], fo[:])

    nc.compile()
    return nc


def _celu64(z):
    return np.where(z > 0, z, ALPHA * np.expm1(np.minimum(z, 0) / ALPHA))


def _bf16_round(x):
    import ml_dtypes
    return np.asarray(x, np.float32).astype(ml_dtypes.bfloat16).astype(np.float64)


def kernel(fullaev, species, W0, b0, W1, b1, W2, b2, W3, b3):
    import ml_dtypes
    from concourse import bass_utils, mybir

    fullaev = np.ascontiguousarray(np.asarray(fullaev, dtype=np.float32))
    species = np.asarray(species, dtype=np.int32)
    Ws = [np.asarray(w, dtype=np.float32) for w in (W0, W1, W2, W3)]
    bs = [np.asarray(b, dtype=np.float32) for b in (b0, b1, b2, b3)]

    # --- species grouping: per-core slot assignment ---------------------
    ids = [np.where(species == s)[0] for s in range(N_SPECIES)]
    n = [len(i) for i in ids]
    G = [-(-n[s] // N_CORES) if n[s] else 0 for s in range(N_SPECIES)]
    S = sum(G)
    key = (tuple(G), X_FP8)
    if key not in _progs:
        _progs[key] = _build_program(G, S)
    nc = _progs[key]

    tiles = _tiles_for_groups(G)
    batches = _batches_for_tiles(tiles)
    nbatches = len(batches)
    WB0 = WPS * N_SPECIES
    CB0 = CPS * N_SPECIES
    WCOLS = WB0 + 128 * nbatches
    CCOLS = CB0 + 2 * nbatches

    # --- fold constants (float64, with bf16-rounded weights) ------------
    cpack = np.zeros((128, CCOLS), np.float32)
    wpack = np.zeros((128, WCOLS), np.float32)
    c3 = np.zeros(N_SPECIES)
    K0 = np.zeros(N_SPECIES)
    c1s = {}
    for s in range(N_SPECIES):
        w1, w2, w3 = (_bf16_round(Ws[l][s]) for l in (1, 2, 3))
        bb0, bb1, bb2, bb3 = (b[s].astype(np.float64) for b in bs)
        c1 = bb1 + w1 @ bb0
        c1s[s] = c1
        c3[s] = bb3[0] + w3[0] @ bb2
        # device contribution of a dummy (zero-AEV) atom, excluding c3
        y0d = _celu64(bb0) - bb0
        y1d = _celu64(w1 @ y0d + c1)
        y2d = _celu64(w2 @ y1d + bb2) - bb2
        K0[s] = w3[0] @ y2d

        cb = s * CPS
        cpack[:, cb + 0] = 10.0 * bb0[:128] + LNA
        cpack[:, cb + 1] = -bb0[:128]
        cpack[:, cb + 2] = 10.0 * c1 + LNA
        cpack[:, cb + 3] = c1
        cpack[:96, cb + 4] = 10.0 * bb2 + LNA
        cpack[:96, cb + 5] = -bb2

        wb = s * WPS
        for fc in range(3):
            blk = Ws[0][s][:, fc * 128:(fc + 1) * 128].T  # [128in, 160out]
            wpack[:, wb + fc * 160: wb + fc * 160 + 160] = blk
        wpack[:, wb + 480: wb + 608] = Ws[1][s][:, :128].T
        wpack[:, wb + 608: wb + 704] = Ws[2][s].T
        wpack[:96, wb + 704] = Ws[3][s][0, :]

    for bi, batch in enumerate(batches):
        for j, ti in enumerate(batch):
            s = tiles[ti][0]
            b0b = bs[0][s].astype(np.float64)[128:]
            cpack[32 * j: 32 * j + 32, CB0 + 2 * bi] = 10.0 * b0b + LNA
            cpack[32 * j: 32 * j + 32, CB0 + 2 * bi + 1] = -b0b
            wpack[32 * j: 32 * j + 32,
                  WB0 + 128 * bi: WB0 + 128 * (bi + 1)] = Ws[1][s][:, 128:].T

    wpack_b = wpack.astype(ml_dtypes.bfloat16)
    x_np_dtype = mybir.dt.np(mybir.dt.float8e4 if X_FP8 else mybir.dt.bfloat16)

    # --- per-core transposed, species-sorted AEV blocks -----------------
    in_maps = []
    dummy_counts = np.zeros((N_CORES, N_SPECIES), np.int64)
    for cid in range(N_CORES):
        xtc = np.zeros((128, S, 3, 128), np.float32)
        slot0 = 0
        for s in range(N_SPECIES):
            mine = ids[s][cid::N_CORES]
            nr = len(mine)
            dummy_counts[cid, s] = G[s] - nr
            if nr:
                g = fullaev[:, mine, :]               # [128, nr, 384]
                t = g.transpose(2, 1, 0)              # [384, nr, 128]
                xtc[:, slot0: slot0 + nr, :, :] = (
                    t.reshape(3, 128, nr, 128).transpose(1, 2, 0, 3)
                )
            slot0 += G[s]
        xq = xtc.reshape(128, S * 384).astype(x_np_dtype)
        in_maps.append({"xt": xq, "wp": wpack_b, "cp": cpack})

    if TRACE:
        _maybe_register_ntff_hook()
    res = bass_utils.run_bass_kernel_spmd(
        nc, in_maps, core_ids=list(range(N_CORES)), trace=TRACE
    )
    LAST["exec_time_ns"] = res.exec_time_ns
    LAST["trace"] = res.instructions_and_trace[1] if res.instructions_and_trace else None

    out = np.zeros(128, np.float64)
    for cid in range(N_CORES):
        out += res.results[cid]["yo"][0].astype(np.float64)
    for s in range(N_SPECIES):
        out += n[s] * c3[s] - dummy_counts[:, s].sum() * K0[s]
    return out.astype(np.float32)


# revision 19
# speedup vs baseline: 1.2065x; 1.0622x over previous
"""ANI-style per-species MLP (384->160->128->96->1, CELU) over [B=128, A=512]
atoms with species routing, atom-summed to [B]. 8-core SPMD Trainium2 kernel.

v2: atom-parallel sharding as before (species-grouped atoms dealt round-robin
to 8 cores, zero-AEV dummy padding corrected on host), with:
  - AEV streamed as fp8e4m3 (rhs) against bf16 weights (lhsT): 4x less DMA.
  - bf16 activations in SBUF; f32 PSUM accumulate.
  - celu split: V1 = exp(ACT) + clamp(DVE ts, bf16 4x mode) + stt(DVE);
    V2 (layer1) = exp+relu on ACT + clamp + tensor_add(DVE 2x) to balance
    ACT vs DVE occupancy.
  - L0's 32-wide output chunk for the 4 tiles of a batch lands in ONE psum
    bank at partition offsets 32j -> one merged celu instead of 4.
  - software-pipelined emission B(k) -> A(k+1) -> C(k) so the PE queue never
    waits on celu latency; PSUM pools shared (4 + 3 + 1 banks).
"""

import os
import sys

import numpy as np

try:
    import concourse  # noqa: F401
except ImportError:
    sys.path.insert(0, "/opt/trn_rl_repo")

N_CORES = 8
B, A, FEAT = 128, 512, 384
N_SPECIES = 4
H0, H1, H2 = 160, 128, 96
ALPHA = 0.1
LNA = float(np.log(ALPHA))

WPS = 3 * 160 + 128 + 96 + 1  # 705 weight-pack columns per species
CPS = 6                       # constant-pack columns per species

X_FP8 = bool(int(os.environ.get("BASSNN_X_FP8", "1")))

TRACE = bool(int(os.environ.get("BASSNN_TRACE", "0")))
LAST = {}

_progs = {}


def _maybe_register_ntff_hook():
    try:
        import types

        import antenv
        from antenv import axon_hooks  # noqa: F401
        return
    except ImportError:
        pass
    try:
        import types

        import antenv
        from trn_agent_boot.trn_boot import _ntff_profile_via_ctypes

        mod = types.ModuleType("antenv.axon_hooks")
        holder = [None]
        mod.set_axon_ntff_profile_hook = lambda h: holder.__setitem__(0, h)
        mod.get_axon_ntff_profile_hook = lambda: holder[0]
        sys.modules["antenv.axon_hooks"] = mod
        antenv.axon_hooks = mod
        mod.set_axon_ntff_profile_hook(
            _ntff_profile_via_ctypes("/opt/axon/libaxon_pjrt.so")
        )
    except Exception:
        pass


def _tiles_for_groups(G):
    """Per-species padded group sizes -> list of (species, slot0, n_atoms)."""
    tiles = []
    slot0 = 0
    for s, g in enumerate(G):
        a = 0
        while a < g:
            na = 4 if g - a >= 4 else g - a
            tiles.append((s, slot0 + a, na))
            a += na
        slot0 += g
    return tiles


def _batches_for_tiles(tiles):
    return [list(range(b, min(b + 4, len(tiles)))) for b in range(0, len(tiles), 4)]


def _build_program(G, S):
    import concourse.bass as bass  # noqa: F401
    import concourse.tile as tile
    from concourse import bacc, mybir

    F32 = mybir.dt.float32
    BF16 = mybir.dt.bfloat16
    X8 = mybir.dt.float8e4 if X_FP8 else BF16
    EXP = mybir.ActivationFunctionType.Exp
    RELU = mybir.ActivationFunctionType.Relu
    MIN = mybir.AluOpType.min
    MAX = mybir.AluOpType.max
    ADD = mybir.AluOpType.add
    SUB = mybir.AluOpType.subtract

    tiles = _tiles_for_groups(G)
    ntiles = len(tiles)
    batches = _batches_for_tiles(tiles)
    nbatches = len(batches)
    WB0 = WPS * N_SPECIES
    CB0 = CPS * N_SPECIES
    WCOLS = WB0 + 128 * nbatches
    CCOLS = CB0 + 2 * nbatches
    # batch k covers consecutive slots [bslot0[k], bslot0[k] + bna[k])
    bslot0 = [tiles[bt[0]][1] for bt in batches]
    bna = [sum(tiles[ti][2] for ti in bt) for bt in batches]
    XB = max(bna) * 384

    nc = bacc.Bacc("TRN2", target_bir_lowering=False, debug=False,
                   num_devices=N_CORES)
    xt = nc.dram_tensor("xt", [128, S * 384], X8, kind="ExternalInput").ap()
    wp = nc.dram_tensor("wp", [128, WCOLS], BF16, kind="ExternalInput").ap()
    cp = nc.dram_tensor("cp", [128, CCOLS], F32, kind="ExternalInput").ap()
    yo = nc.dram_tensor("yo", [1, 512], F32, kind="ExternalOutput").ap()

    with tile.TileContext(nc) as tc:
        with (
            tc.tile_pool(name="wpool", bufs=1) as wpool,
            tc.tile_pool(name="cpool", bufs=1) as cpool,
            tc.tile_pool(name="xpool", bufs=3) as xpool,
            tc.tile_pool(name="epool", bufs=6) as epool,
            tc.tile_pool(name="tpool", bufs=6) as tpool,
            tc.tile_pool(name="rpool", bufs=3) as rpool,
            tc.tile_pool(name="y0apool", bufs=6) as y0apool,
            tc.tile_pool(name="y0bpool", bufs=2) as y0bpool,
            tc.tile_pool(name="y1pool", bufs=5) as y1pool,
            tc.tile_pool(name="y2pool", bufs=6) as y2pool,
            tc.tile_pool(name="opool", bufs=1) as opool,
            tc.tile_pool(name="ppa", bufs=2, space="PSUM") as ppa,
            tc.tile_pool(name="pp1", bufs=2, space="PSUM") as pp1,
            tc.tile_pool(name="ppb0", bufs=1, space="PSUM") as ppb0,
            tc.tile_pool(name="ppb2", bufs=2, space="PSUM") as ppb2,
            tc.tile_pool(name="pp3", bufs=1, space="PSUM") as pp3,
        ):
            # weights/constants on the gpsimd DMA queue, x batches on sync:
            # descriptor generation and transfers overlap. The first species'
            # weight block and the constants go first so the opening matmuls
            # and celus are not gated on the full weight pack (subtile deps).
            w = wpool.tile([128, WCOLS], BF16)
            c = cpool.tile([128, CCOLS], F32)
            nc.gpsimd.dma_start(c[:], cp[:])
            s_first = tiles[0][0]
            wA, wB = WPS * s_first, WPS * s_first + WPS
            nc.gpsimd.dma_start(w[:, wA:wB], wp[:, wA:wB])
            if wA > 0:
                nc.gpsimd.dma_start(w[:, 0:wA], wp[:, 0:wA])
            nc.gpsimd.dma_start(w[:, wB:WCOLS], wp[:, wB:WCOLS])

            def wcol(s, off, n):
                return w[:, s * WPS + off: s * WPS + off + n]

            def ccol(s, k, parts):
                return c[0:parts, s * CPS + k: s * CPS + k + 1]

            p3 = pp3.tile([1, 512], F32)

            def celu_v1(y_ap, p_ap, e_ap, t_ap, ebias, mbias):
                # y = (P max mbias) + min(alpha*e^(10P+ebias) - alpha, 0)
                nc.scalar.activation(e_ap, p_ap, EXP, bias=ebias, scale=10.0)
                nc.vector.tensor_scalar(t_ap, e_ap, ALPHA, 0.0, SUB, MIN)
                nc.vector.scalar_tensor_tensor(y_ap, p_ap, mbias, t_ap,
                                               MAX, ADD)

            def celu_v2(y_ap, p_ap, e_ap, t_ap, r_ap, ebias, rbias):
                # y = relu(P + rbias) + min(alpha*e^(10P+ebias) - alpha, 0)
                nc.scalar.activation(e_ap, p_ap, EXP, bias=ebias, scale=10.0)
                nc.scalar.activation(r_ap, p_ap, RELU, bias=rbias, scale=1.0)
                nc.vector.tensor_scalar(t_ap, e_ap, ALPHA, 0.0, SUB, MIN)
                nc.vector.tensor_add(y_ap, r_ap, t_ap)

            xts = {}

            def dma_batch(k, split_first=False):
                t = xpool.tile([128, XB], X8, name="xts")
                o = bslot0[k] * 384
                if split_first:
                    n0 = tiles[batches[k][0]][2] * 384
                    nc.sync.dma_start(t[:, 0:n0], xt[:, o: o + n0])
                    nc.sync.dma_start(t[:, n0: bna[k] * 384],
                                      xt[:, o + n0: o + bna[k] * 384])
                else:
                    nc.sync.dma_start(t[:, 0: bna[k] * 384],
                                      xt[:, o: o + bna[k] * 384])
                xts[k] = t

            y0as = {}
            y0bs = {}
            y1s = {}
            y2s = {}
            p0bs = {}

            def l0_mm_thunks(k):
                """Batch k's L0 matmuls as 12 thunks (2 mms each)."""
                batch = batches[k]
                xv = xts[k].rearrange("p (a f m) -> p a f m",
                                      a=XB // 384, f=3, m=128)
                p0b = ppb0.tile([128, 512], F32, tag="pb", name="p0b")
                p0bs[k] = p0b
                p0as = {}
                thunks = []
                for j, ti in enumerate(batch):
                    s, a0, na = tiles[ti]
                    N = na * 128
                    aoff = a0 - bslot0[k]
                    p0a = ppa.tile([128, 512], F32, tag="pa", name="p0a")
                    p0as[ti] = p0a

                    def mk(j=j, s=s, na=na, N=N, aoff=aoff, p0a=p0a, fc=0):
                        rhs = xv[:, aoff: aoff + na, fc, :]
                        nc.tensor.matmul(p0a[:, 0:N],
                                         wcol(s, fc * 160, 128),
                                         rhs, start=(fc == 0), stop=(fc == 2))
                        nc.tensor.matmul(p0b[32 * j: 32 * j + 32, 0:N],
                                         wcol(s, fc * 160 + 128, 32),
                                         rhs, start=(fc == 0), stop=(fc == 2),
                                         tile_position=(0, 32 * j))
                    for fc in range(3):
                        thunks.append(
                            lambda mk=mk, fc=fc: mk(fc=fc))
                return thunks, p0as

            def l0a_celu(k, j, p0as):
                ti = batches[k][j]
                s, a0, na = tiles[ti]
                N = na * 128
                ea = epool.tile([128, 512], BF16, name="ea")
                ta = tpool.tile([128, 512], BF16, name="ta")
                y0a = y0apool.tile([128, 512], BF16, name="y0a")
                celu_v1(y0a[:, 0:N], p0as[ti][:, 0:N], ea[:, 0:N],
                        ta[:, 0:N], ccol(s, 0, 128), ccol(s, 1, 128))
                y0as[ti] = y0a

            def l0b_celu(k):
                L = len(batches[k])
                em = epool.tile([128, 512], BF16, name="em")
                tm = tpool.tile([128, 512], BF16, name="tm")
                y0b = y0bpool.tile([128, 512], BF16, name="y0b")
                P = 32 * L
                celu_v1(y0b[0:P, :], p0bs[k][0:P, :], em[0:P, :], tm[0:P, :],
                        c[0:P, CB0 + 2 * k: CB0 + 2 * k + 1],
                        c[0:P, CB0 + 2 * k + 1: CB0 + 2 * k + 2])
                y0bs[k] = y0b

            def steady(k):
                """B(k) with A(k+1)'s L0 matmuls interleaved, then C(k)."""
                batch = batches[k]
                y0b = y0bs[k]
                if k + 2 < nbatches:
                    dma_batch(k + 2)
                p1s = {}
                for j, ti in enumerate(batch):
                    s, a0, na = tiles[ti]
                    N = na * 128
                    p1 = pp1.tile([128, 512], F32, tag="p1", name="p1")
                    nc.tensor.matmul(p1[:, 0:N], wcol(s, 480, 128),
                                     y0as[ti][:, 0:N], start=True, stop=False)
                    nc.tensor.matmul(
                        p1[:, 0:N],
                        w[32 * j: 32 * j + 32,
                          WB0 + 128 * k: WB0 + 128 * (k + 1)],
                        y0b[32 * j: 32 * j + 32, 0:N],
                        start=False, stop=True,
                        tile_position=(32 * j, 0))
                    p1s[ti] = p1
                for j, ti in enumerate(batch):
                    s, a0, na = tiles[ti]
                    N = na * 128
                    e1 = epool.tile([128, 512], BF16, name="e1")
                    t1 = tpool.tile([128, 512], BF16, name="t1")
                    r1 = rpool.tile([128, 512], BF16, name="r1")
                    y1 = y1pool.tile([128, 512], BF16, name="y1")
                    celu_v2(y1[:, 0:N], p1s[ti][:, 0:N], e1[:, 0:N],
                            t1[:, 0:N], r1[:, 0:N],
                            ccol(s, 2, 128), ccol(s, 3, 128))
                    y1s[ti] = y1
                if k + 1 < nbatches:
                    thunks, p0as_next = l0_mm_thunks(k + 1)
                else:
                    thunks, p0as_next = [], None
                chunks = [thunks[3 * i: 3 * i + 3] for i in range(4)]
                nnext = len(batches[k + 1]) if p0as_next is not None else 0
                for j, ti in enumerate(batch):
                    for th in chunks[j]:
                        th()
                    if p0as_next is not None and j < nnext:
                        l0a_celu(k + 1, j, p0as_next)
                    if j == len(batch) - 1:
                        # all of next batch's L0 matmuls are in by now:
                        # finish its celus so the merged-L0b stt lands well
                        # before the next cycle's L1b matmuls need it.
                        for ch in chunks[len(batch):]:
                            for th in ch:
                                th()
                        if p0as_next is not None:
                            for jj in range(len(batch), nnext):
                                l0a_celu(k + 1, jj, p0as_next)
                            l0b_celu(k + 1)
                    s, a0, na = tiles[ti]
                    N = na * 128
                    p2 = ppb2.tile([96, 512], F32, tag="p2", name="p2")
                    nc.tensor.matmul(p2[:, 0:N], wcol(s, 608, 96),
                                     y1s[ti][:, 0:N], start=True, stop=True)
                    e2 = epool.tile([96, 512], BF16, name="e2")
                    t2 = tpool.tile([96, 512], BF16, name="t2")
                    y2 = y2pool.tile([96, 512], BF16, name="y2")
                    celu_v1(y2[:, 0:N], p2[:, 0:N], e2[:, 0:N], t2[:, 0:N],
                            ccol(s, 4, 96), ccol(s, 5, 96))
                    y2s[ti] = y2
                for j, ti in enumerate(batch):
                    s, a0, na = tiles[ti]
                    N = na * 128
                    nc.tensor.matmul(p3[0:1, 0:N],
                                     wcol(s, 704, 1)[0:96, :],
                                     y2s[ti][0:96, 0:N],
                                     start=(ti == 0), stop=(ti == ntiles - 1),
                                     skip_group_check=True)

            dma_batch(0, split_first=True)
            if nbatches > 1:
                dma_batch(1)
            thunks0, p0as0 = l0_mm_thunks(0)
            for th in thunks0:
                th()
            for j in range(len(batches[0])):
                l0a_celu(0, j, p0as0)
            l0b_celu(0)
            for k in range(nbatches):
                steady(k)

            t3 = opool.tile([1, 512], F32)
            nc.scalar.copy(t3[:], p3[:])
            nc.sync.dma_start(yo[:], t3[:])

    nc.compile()
    return nc


def _celu64(z):
    return np.where(z > 0, z, ALPHA * np.expm1(np.minimum(z, 0) / ALPHA))


def _bf16_round(x):
    import ml_dtypes
    return np.asarray(x, np.float32).astype(ml_dtypes.bfloat16).astype(np.float64)


def kernel(fullaev, species, W0, b0, W1, b1, W2, b2, W3, b3):
    import ml_dtypes
    from concourse import bass_utils, mybir

    fullaev = np.ascontiguousarray(np.asarray(fullaev, dtype=np.float32))
    species = np.asarray(species, dtype=np.int32)
    Ws = [np.asarray(w, dtype=np.float32) for w in (W0, W1, W2, W3)]
    bs = [np.asarray(b, dtype=np.float32) for b in (b0, b1, b2, b3)]

    # --- species grouping: per-core slot assignment ---------------------
    ids = [np.where(species == s)[0] for s in range(N_SPECIES)]
    n = [len(i) for i in ids]
    G = [-(-n[s] // N_CORES) if n[s] else 0 for s in range(N_SPECIES)]
    S = sum(G)
    key = (tuple(G), X_FP8)
    if key not in _progs:
        _progs[key] = _build_program(G, S)
    nc = _progs[key]

    tiles = _tiles_for_groups(G)
    batches = _batches_for_tiles(tiles)
    nbatches = len(batches)
    WB0 = WPS * N_SPECIES
    CB0 = CPS * N_SPECIES
    WCOLS = WB0 + 128 * nbatches
    CCOLS = CB0 + 2 * nbatches

    # --- fold constants (float64, with bf16-rounded weights) ------------
    cpack = np.zeros((128, CCOLS), np.float32)
    wpack = np.zeros((128, WCOLS), np.float32)
    c3 = np.zeros(N_SPECIES)
    K0 = np.zeros(N_SPECIES)
    c1s = {}
    for s in range(N_SPECIES):
        w1, w2, w3 = (_bf16_round(Ws[l][s]) for l in (1, 2, 3))
        bb0, bb1, bb2, bb3 = (b[s].astype(np.float64) for b in bs)
        c1 = bb1 + w1 @ bb0
        c1s[s] = c1
        c3[s] = bb3[0] + w3[0] @ bb2
        # device contribution of a dummy (zero-AEV) atom, excluding c3
        y0d = _celu64(bb0) - bb0
        y1d = _celu64(w1 @ y0d + c1)
        y2d = _celu64(w2 @ y1d + bb2) - bb2
        K0[s] = w3[0] @ y2d

        cb = s * CPS
        cpack[:, cb + 0] = 10.0 * bb0[:128] + LNA
        cpack[:, cb + 1] = -bb0[:128]
        cpack[:, cb + 2] = 10.0 * c1 + LNA
        cpack[:, cb + 3] = c1
        cpack[:96, cb + 4] = 10.0 * bb2 + LNA
        cpack[:96, cb + 5] = -bb2

        wb = s * WPS
        for fc in range(3):
            blk = Ws[0][s][:, fc * 128:(fc + 1) * 128].T  # [128in, 160out]
            wpack[:, wb + fc * 160: wb + fc * 160 + 160] = blk
        wpack[:, wb + 480: wb + 608] = Ws[1][s][:, :128].T
        wpack[:, wb + 608: wb + 704] = Ws[2][s].T
        wpack[:96, wb + 704] = Ws[3][s][0, :]

    for bi, batch in enumerate(batches):
        for j, ti in enumerate(batch):
            s = tiles[ti][0]
            b0b = bs[0][s].astype(np.float64)[128:]
            cpack[32 * j: 32 * j + 32, CB0 + 2 * bi] = 10.0 * b0b + LNA
            cpack[32 * j: 32 * j + 32, CB0 + 2 * bi + 1] = -b0b
            wpack[32 * j: 32 * j + 32,
                  WB0 + 128 * bi: WB0 + 128 * (bi + 1)] = Ws[1][s][:, 128:].T

    wpack_b = wpack.astype(ml_dtypes.bfloat16)
    x_np_dtype = mybir.dt.np(mybir.dt.float8e4 if X_FP8 else mybir.dt.bfloat16)

    # --- per-core transposed, species-sorted AEV blocks -----------------
    in_maps = []
    dummy_counts = np.zeros((N_CORES, N_SPECIES), np.int64)
    for cid in range(N_CORES):
        xtc = np.zeros((128, S, 3, 128), np.float32)
        slot0 = 0
        for s in range(N_SPECIES):
            mine = ids[s][cid::N_CORES]
            nr = len(mine)
            dummy_counts[cid, s] = G[s] - nr
            if nr:
                g = fullaev[:, mine, :]               # [128, nr, 384]
                t = g.transpose(2, 1, 0)              # [384, nr, 128]
                xtc[:, slot0: slot0 + nr, :, :] = (
                    t.reshape(3, 128, nr, 128).transpose(1, 2, 0, 3)
                )
            slot0 += G[s]
        xq = xtc.reshape(128, S * 384).astype(x_np_dtype)
        in_maps.append({"xt": xq, "wp": wpack_b, "cp": cpack})

    if TRACE:
        _maybe_register_ntff_hook()
    res = bass_utils.run_bass_kernel_spmd(
        nc, in_maps, core_ids=list(range(N_CORES)), trace=TRACE
    )
    LAST["exec_time_ns"] = res.exec_time_ns
    LAST["trace"] = res.instructions_and_trace[1] if res.instructions_and_trace else None

    out = np.zeros(128, np.float64)
    for cid in range(N_CORES):
        out += (res.results[cid]["yo"][0].astype(np.float64)
                .reshape(4, 128).sum(axis=0))
    for s in range(N_SPECIES):
        out += n[s] * c3[s] - dummy_counts[:, s].sum() * K0[s]
    return out.astype(np.float32)


# revision 22
# speedup vs baseline: 1.2215x; 1.0124x over previous
"""ANI-style per-species MLP (384->160->128->96->1, CELU) over [B=128, A=512]
atoms with species routing, atom-summed to [B]. 8-core SPMD Trainium2 kernel.

v2: atom-parallel sharding as before (species-grouped atoms dealt round-robin
to 8 cores, zero-AEV dummy padding corrected on host), with:
  - AEV streamed as fp8e4m3 (rhs) against bf16 weights (lhsT): 4x less DMA.
  - bf16 activations in SBUF; f32 PSUM accumulate.
  - celu split: V1 = exp(ACT) + clamp(DVE ts, bf16 4x mode) + stt(DVE);
    V2 (layer1) = exp+relu on ACT + clamp + tensor_add(DVE 2x) to balance
    ACT vs DVE occupancy.
  - L0's 32-wide output chunk for the 4 tiles of a batch lands in ONE psum
    bank at partition offsets 32j -> one merged celu instead of 4.
  - software-pipelined emission B(k) -> A(k+1) -> C(k) so the PE queue never
    waits on celu latency; PSUM pools shared (4 + 3 + 1 banks).
"""

import os
import sys

import numpy as np

try:
    import concourse  # noqa: F401
except ImportError:
    sys.path.insert(0, "/opt/trn_rl_repo")

N_CORES = 8
B, A, FEAT = 128, 512, 384
N_SPECIES = 4
H0, H1, H2 = 160, 128, 96
ALPHA = 0.1
LNA = float(np.log(ALPHA))

WPS = 3 * 160 + 128 + 96 + 1  # 705 weight-pack columns per species
CPS = 6                       # constant-pack columns per species

X_FP8 = bool(int(os.environ.get("BASSNN_X_FP8", "1")))
L0_DP = bool(int(os.environ.get("BASSNN_L0_DP", "0")))

TRACE = bool(int(os.environ.get("BASSNN_TRACE", "0")))
LAST = {}

_progs = {}


def _maybe_register_ntff_hook():
    try:
        import types

        import antenv
        from antenv import axon_hooks  # noqa: F401
        return
    except ImportError:
        pass
    try:
        import types

        import antenv
        from trn_agent_boot.trn_boot import _ntff_profile_via_ctypes

        mod = types.ModuleType("antenv.axon_hooks")
        holder = [None]
        mod.set_axon_ntff_profile_hook = lambda h: holder.__setitem__(0, h)
        mod.get_axon_ntff_profile_hook = lambda: holder[0]
        sys.modules["antenv.axon_hooks"] = mod
        antenv.axon_hooks = mod
        mod.set_axon_ntff_profile_hook(
            _ntff_profile_via_ctypes("/opt/axon/libaxon_pjrt.so")
        )
    except Exception:
        pass


def _tiles_for_groups(G):
    """Per-species padded group sizes -> list of (species, slot0, n_atoms)."""
    tiles = []
    slot0 = 0
    for s, g in enumerate(G):
        a = 0
        while a < g:
            na = 4 if g - a >= 4 else g - a
            tiles.append((s, slot0 + a, na))
            a += na
        slot0 += g
    return tiles


def _batches_for_tiles(tiles):
    return [list(range(b, min(b + 4, len(tiles)))) for b in range(0, len(tiles), 4)]


def _build_program(G, S):
    import concourse.bass as bass  # noqa: F401
    import concourse.tile as tile
    from concourse import bacc, mybir

    F32 = mybir.dt.float32
    BF16 = mybir.dt.bfloat16
    X8 = mybir.dt.float8e4 if X_FP8 else BF16
    EXP = mybir.ActivationFunctionType.Exp
    RELU = mybir.ActivationFunctionType.Relu
    MIN = mybir.AluOpType.min
    MAX = mybir.AluOpType.max
    ADD = mybir.AluOpType.add
    SUB = mybir.AluOpType.subtract

    tiles = _tiles_for_groups(G)
    ntiles = len(tiles)
    batches = _batches_for_tiles(tiles)
    nbatches = len(batches)
    WB0 = WPS * N_SPECIES
    CB0 = CPS * N_SPECIES
    WCOLS = WB0 + 128 * nbatches
    CCOLS = CB0 + 2 * nbatches
    # batch k covers consecutive slots [bslot0[k], bslot0[k] + bna[k])
    bslot0 = [tiles[bt[0]][1] for bt in batches]
    bna = [sum(tiles[ti][2] for ti in bt) for bt in batches]
    XB = max(bna) * 384

    nc = bacc.Bacc("TRN2", target_bir_lowering=False, debug=False,
                   num_devices=N_CORES)
    xt = nc.dram_tensor("xt", [128, S * 384], X8, kind="ExternalInput").ap()
    wp = nc.dram_tensor("wp", [128, WCOLS], BF16, kind="ExternalInput").ap()
    cp = nc.dram_tensor("cp", [128, CCOLS], F32, kind="ExternalInput").ap()
    yo = nc.dram_tensor("yo", [1, 512], F32, kind="ExternalOutput").ap()

    with tile.TileContext(nc) as tc:
        with (
            tc.tile_pool(name="wpool", bufs=1) as wpool,
            tc.tile_pool(name="cpool", bufs=1) as cpool,
            tc.tile_pool(name="xpool", bufs=3) as xpool,
            tc.tile_pool(name="epool", bufs=6) as epool,
            tc.tile_pool(name="tpool", bufs=6) as tpool,
            tc.tile_pool(name="rpool", bufs=3) as rpool,
            tc.tile_pool(name="y0apool", bufs=6) as y0apool,
            tc.tile_pool(name="y0bpool", bufs=2) as y0bpool,
            tc.tile_pool(name="y1pool", bufs=5) as y1pool,
            tc.tile_pool(name="y2pool", bufs=6) as y2pool,
            tc.tile_pool(name="opool", bufs=1) as opool,
            tc.tile_pool(name="ppa", bufs=2, space="PSUM") as ppa,
            tc.tile_pool(name="pp1", bufs=2, space="PSUM") as pp1,
            tc.tile_pool(name="ppb0", bufs=1, space="PSUM") as ppb0,
            tc.tile_pool(name="ppb2", bufs=2, space="PSUM") as ppb2,
            tc.tile_pool(name="pp3", bufs=1, space="PSUM") as pp3,
        ):
            # weights/constants on the gpsimd DMA queue, x batches on sync:
            # descriptor generation and transfers overlap. The first species'
            # weight block and the constants go first so the opening matmuls
            # and celus are not gated on the full weight pack (subtile deps).
            w = wpool.tile([128, WCOLS], BF16)
            c = cpool.tile([128, CCOLS], F32)
            nc.gpsimd.dma_start(c[:], cp[:])
            s_first = tiles[0][0]
            wA, wB = WPS * s_first, WPS * s_first + WPS
            nc.gpsimd.dma_start(w[:, wA:wB], wp[:, wA:wB])
            if wA > 0:
                nc.gpsimd.dma_start(w[:, 0:wA], wp[:, 0:wA])
            nc.gpsimd.dma_start(w[:, wB:WCOLS], wp[:, wB:WCOLS])

            def wcol(s, off, n):
                return w[:, s * WPS + off: s * WPS + off + n]

            def ccol(s, k, parts):
                return c[0:parts, s * CPS + k: s * CPS + k + 1]

            p3 = pp3.tile([1, 512], F32)

            def celu_v1(y_ap, p_ap, e_ap, t_ap, ebias, mbias):
                # y = (P max mbias) + min(alpha*e^(10P+ebias) - alpha, 0)
                nc.scalar.activation(e_ap, p_ap, EXP, bias=ebias, scale=10.0)
                nc.vector.tensor_scalar(t_ap, e_ap, ALPHA, 0.0, SUB, MIN)
                nc.vector.scalar_tensor_tensor(y_ap, p_ap, mbias, t_ap,
                                               MAX, ADD)

            def celu_v2(y_ap, p_ap, e_ap, t_ap, r_ap, ebias, rbias):
                # y = relu(P + rbias) + min(alpha*e^(10P+ebias) - alpha, 0)
                nc.scalar.activation(e_ap, p_ap, EXP, bias=ebias, scale=10.0)
                nc.scalar.activation(r_ap, p_ap, RELU, bias=rbias, scale=1.0)
                nc.vector.tensor_scalar(t_ap, e_ap, ALPHA, 0.0, SUB, MIN)
                nc.vector.tensor_add(y_ap, r_ap, t_ap)

            xts = {}

            def dma_batch(k, split_first=False):
                t = xpool.tile([128, XB], X8, name="xts")
                o = bslot0[k] * 384
                if split_first:
                    n0 = tiles[batches[k][0]][2] * 384
                    nc.sync.dma_start(t[:, 0:n0], xt[:, o: o + n0])
                    nc.sync.dma_start(t[:, n0: bna[k] * 384],
                                      xt[:, o + n0: o + bna[k] * 384])
                else:
                    nc.sync.dma_start(t[:, 0: bna[k] * 384],
                                      xt[:, o: o + bna[k] * 384])
                xts[k] = t

            y0as = {}
            y0bs = {}
            y1s = {}
            y2s = {}
            p0bs = {}

            def l0_mm_thunks(k):
                """Batch k's L0 matmuls as 12 thunks (2 mms each)."""
                batch = batches[k]
                xv = xts[k].rearrange("p (a f m) -> p a f m",
                                      a=XB // 384, f=3, m=128)
                p0b = ppb0.tile([128, 512], F32, tag="pb", name="p0b")
                p0bs[k] = p0b
                p0as = {}
                thunks = []
                for j, ti in enumerate(batch):
                    s, a0, na = tiles[ti]
                    N = na * 128
                    aoff = a0 - bslot0[k]
                    p0a = ppa.tile([128, 512], F32, tag="pa", name="p0a")
                    p0as[ti] = p0a

                    pm = (mybir.MatmulPerfMode.DoublePixel
                          if (L0_DP and X_FP8) else None)

                    def mk(j=j, s=s, na=na, N=N, aoff=aoff, p0a=p0a,
                           pm=pm, fc=0):
                        rhs = xv[:, aoff: aoff + na, fc, :]
                        nc.tensor.matmul(p0a[:, 0:N],
                                         wcol(s, fc * 160, 128),
                                         rhs, start=(fc == 0), stop=(fc == 2),
                                         perf_mode=pm)
                        nc.tensor.matmul(p0b[32 * j: 32 * j + 32, 0:N],
                                         wcol(s, fc * 160 + 128, 32),
                                         rhs, start=(fc == 0), stop=(fc == 2),
                                         tile_position=(0, 32 * j),
                                         perf_mode=pm)
                    for fc in range(3):
                        thunks.append(
                            lambda mk=mk, fc=fc: mk(fc=fc))
                return thunks, p0as

            def l0a_celu(k, j, p0as):
                ti = batches[k][j]
                s, a0, na = tiles[ti]
                N = na * 128
                ea = epool.tile([128, 512], BF16, name="ea")
                ta = tpool.tile([128, 512], BF16, name="ta")
                y0a = y0apool.tile([128, 512], BF16, name="y0a")
                celu_v1(y0a[:, 0:N], p0as[ti][:, 0:N], ea[:, 0:N],
                        ta[:, 0:N], ccol(s, 0, 128), ccol(s, 1, 128))
                y0as[ti] = y0a

            def l0b_celu(k):
                L = len(batches[k])
                em = epool.tile([128, 512], BF16, name="em")
                tm = tpool.tile([128, 512], BF16, name="tm")
                y0b = y0bpool.tile([128, 512], BF16, name="y0b")
                P = 32 * L
                celu_v1(y0b[0:P, :], p0bs[k][0:P, :], em[0:P, :], tm[0:P, :],
                        c[0:P, CB0 + 2 * k: CB0 + 2 * k + 1],
                        c[0:P, CB0 + 2 * k + 1: CB0 + 2 * k + 2])
                y0bs[k] = y0b

            def steady(k):
                """B(k) with A(k+1)'s L0 matmuls interleaved, then C(k)."""
                batch = batches[k]
                y0b = y0bs[k]
                if k + 2 < nbatches:
                    dma_batch(k + 2)
                p1s = {}
                for j, ti in enumerate(batch):
                    s, a0, na = tiles[ti]
                    N = na * 128
                    p1 = pp1.tile([128, 512], F32, tag="p1", name="p1")
                    nc.tensor.matmul(p1[:, 0:N], wcol(s, 480, 128),
                                     y0as[ti][:, 0:N], start=True, stop=False)
                    nc.tensor.matmul(
                        p1[:, 0:N],
                        w[32 * j: 32 * j + 32,
                          WB0 + 128 * k: WB0 + 128 * (k + 1)],
                        y0b[32 * j: 32 * j + 32, 0:N],
                        start=False, stop=True,
                        tile_position=(32 * j, 0))
                    p1s[ti] = p1
                for j, ti in enumerate(batch):
                    s, a0, na = tiles[ti]
                    N = na * 128
                    e1 = epool.tile([128, 512], BF16, name="e1")
                    t1 = tpool.tile([128, 512], BF16, name="t1")
                    r1 = rpool.tile([128, 512], BF16, name="r1")
                    y1 = y1pool.tile([128, 512], BF16, name="y1")
                    celu_v2(y1[:, 0:N], p1s[ti][:, 0:N], e1[:, 0:N],
                            t1[:, 0:N], r1[:, 0:N],
                            ccol(s, 2, 128), ccol(s, 3, 128))
                    y1s[ti] = y1
                if k + 1 < nbatches:
                    thunks, p0as_next = l0_mm_thunks(k + 1)
                else:
                    thunks, p0as_next = [], None
                chunks = [thunks[3 * i: 3 * i + 3] for i in range(4)]
                nnext = len(batches[k + 1]) if p0as_next is not None else 0
                for j, ti in enumerate(batch):
                    for th in chunks[j]:
                        th()
                    if p0as_next is not None and j < nnext:
                        l0a_celu(k + 1, j, p0as_next)
                    if j == len(batch) - 1:
                        # all of next batch's L0 matmuls are in by now:
                        # finish its celus so the merged-L0b stt lands well
                        # before the next cycle's L1b matmuls need it.
                        for ch in chunks[len(batch):]:
                            for th in ch:
                                th()
                        if p0as_next is not None:
                            for jj in range(len(batch), nnext):
                                l0a_celu(k + 1, jj, p0as_next)
                            l0b_celu(k + 1)
                    s, a0, na = tiles[ti]
                    N = na * 128
                    p2 = ppb2.tile([96, 512], F32, tag="p2", name="p2")
                    nc.tensor.matmul(p2[:, 0:N], wcol(s, 608, 96),
                                     y1s[ti][:, 0:N], start=True, stop=True)
                    e2 = epool.tile([96, 512], BF16, name="e2")
                    t2 = tpool.tile([96, 512], BF16, name="t2")
                    y2 = y2pool.tile([96, 512], BF16, name="y2")
                    celu_v1(y2[:, 0:N], p2[:, 0:N], e2[:, 0:N], t2[:, 0:N],
                            ccol(s, 4, 96), ccol(s, 5, 96))
                    y2s[ti] = y2
                for j, ti in enumerate(batch):
                    s, a0, na = tiles[ti]
                    N = na * 128
                    nc.tensor.matmul(p3[0:1, 0:N],
                                     wcol(s, 704, 1)[0:96, :],
                                     y2s[ti][0:96, 0:N],
                                     start=(ti == 0), stop=(ti == ntiles - 1),
                                     skip_group_check=True)

            dma_batch(0, split_first=True)
            if nbatches > 1:
                dma_batch(1)
            thunks0, p0as0 = l0_mm_thunks(0)
            for th in thunks0:
                th()
            for j in range(len(batches[0])):
                l0a_celu(0, j, p0as0)
            l0b_celu(0)
            for k in range(nbatches):
                steady(k)

            t3 = opool.tile([1, 512], F32)
            nc.scalar.copy(t3[:], p3[:])
            nc.sync.dma_start(yo[:], t3[:])

    nc.compile()
    return nc


def _celu64(z):
    return np.where(z > 0, z, ALPHA * np.expm1(np.minimum(z, 0) / ALPHA))


def _bf16_round(x):
    import ml_dtypes
    return np.asarray(x, np.float32).astype(ml_dtypes.bfloat16).astype(np.float64)


def kernel(fullaev, species, W0, b0, W1, b1, W2, b2, W3, b3):
    import ml_dtypes
    from concourse import bass_utils, mybir

    fullaev = np.ascontiguousarray(np.asarray(fullaev, dtype=np.float32))
    species = np.asarray(species, dtype=np.int32)
    Ws = [np.asarray(w, dtype=np.float32) for w in (W0, W1, W2, W3)]
    bs = [np.asarray(b, dtype=np.float32) for b in (b0, b1, b2, b3)]

    # --- species grouping: per-core slot assignment ---------------------
    ids = [np.where(species == s)[0] for s in range(N_SPECIES)]
    n = [len(i) for i in ids]
    G = [-(-n[s] // N_CORES) if n[s] else 0 for s in range(N_SPECIES)]
    S = sum(G)
    key = (tuple(G), X_FP8, L0_DP)
    if key not in _progs:
        _progs[key] = _build_program(G, S)
    nc = _progs[key]

    tiles = _tiles_for_groups(G)
    batches = _batches_for_tiles(tiles)
    nbatches = len(batches)
    WB0 = WPS * N_SPECIES
    CB0 = CPS * N_SPECIES
    WCOLS = WB0 + 128 * nbatches
    CCOLS = CB0 + 2 * nbatches

    # --- fold constants (float64, with bf16-rounded weights) ------------
    cpack = np.zeros((128, CCOLS), np.float32)
    wpack = np.zeros((128, WCOLS), np.float32)
    c3 = np.zeros(N_SPECIES)
    K0 = np.zeros(N_SPECIES)
    c1s = {}
    for s in range(N_SPECIES):
        w1, w2, w3 = (_bf16_round(Ws[l][s]) for l in (1, 2, 3))
        bb0, bb1, bb2, bb3 = (b[s].astype(np.float64) for b in bs)
        c1 = bb1 + w1 @ bb0
        c1s[s] = c1
        c3[s] = bb3[0] + w3[0] @ bb2
        # device contribution of a dummy (zero-AEV) atom, excluding c3
        y0d = _celu64(bb0) - bb0
        y1d = _celu64(w1 @ y0d + c1)
        y2d = _celu64(w2 @ y1d + bb2) - bb2
        K0[s] = w3[0] @ y2d

        cb = s * CPS
        cpack[:, cb + 0] = 10.0 * bb0[:128] + LNA
        cpack[:, cb + 1] = -bb0[:128]
        cpack[:, cb + 2] = 10.0 * c1 + LNA
        cpack[:, cb + 3] = c1
        cpack[:96, cb + 4] = 10.0 * bb2 + LNA
        cpack[:96, cb + 5] = -bb2

        wb = s * WPS
        for fc in range(3):
            blk = Ws[0][s][:, fc * 128:(fc + 1) * 128].T  # [128in, 160out]
            wpack[:, wb + fc * 160: wb + fc * 160 + 160] = blk
        wpack[:, wb + 480: wb + 608] = Ws[1][s][:, :128].T
        wpack[:, wb + 608: wb + 704] = Ws[2][s].T
        wpack[:96, wb + 704] = Ws[3][s][0, :]

    for bi, batch in enumerate(batches):
        for j, ti in enumerate(batch):
            s = tiles[ti][0]
            b0b = bs[0][s].astype(np.float64)[128:]
            cpack[32 * j: 32 * j + 32, CB0 + 2 * bi] = 10.0 * b0b + LNA
            cpack[32 * j: 32 * j + 32, CB0 + 2 * bi + 1] = -b0b
            wpack[32 * j: 32 * j + 32,
                  WB0 + 128 * bi: WB0 + 128 * (bi + 1)] = Ws[1][s][:, 128:].T

    wpack_b = wpack.astype(ml_dtypes.bfloat16)
    x_np_dtype = mybir.dt.np(mybir.dt.float8e4 if X_FP8 else mybir.dt.bfloat16)

    # --- per-core transposed, species-sorted AEV blocks -----------------
    in_maps = []
    dummy_counts = np.zeros((N_CORES, N_SPECIES), np.int64)
    for cid in range(N_CORES):
        xtc = np.zeros((128, S, 3, 128), np.float32)
        slot0 = 0
        for s in range(N_SPECIES):
            mine = ids[s][cid::N_CORES]
            nr = len(mine)
            dummy_counts[cid, s] = G[s] - nr
            if nr:
                g = fullaev[:, mine, :]               # [128, nr, 384]
                t = g.transpose(2, 1, 0)              # [384, nr, 128]
                xtc[:, slot0: slot0 + nr, :, :] = (
                    t.reshape(3, 128, nr, 128).transpose(1, 2, 0, 3)
                )
            slot0 += G[s]
        xq = xtc.reshape(128, S * 384).astype(x_np_dtype)
        in_maps.append({"xt": xq, "wp": wpack_b, "cp": cpack})

    if TRACE:
        _maybe_register_ntff_hook()
    res = bass_utils.run_bass_kernel_spmd(
        nc, in_maps, core_ids=list(range(N_CORES)), trace=TRACE
    )
    LAST["exec_time_ns"] = res.exec_time_ns
    LAST["trace"] = res.instructions_and_trace[1] if res.instructions_and_trace else None

    out = np.zeros(128, np.float64)
    for cid in range(N_CORES):
        out += (res.results[cid]["yo"][0].astype(np.float64)
                .reshape(4, 128).sum(axis=0))
    for s in range(N_SPECIES):
        out += n[s] * c3[s] - dummy_counts[:, s].sum() * K0[s]
    return out.astype(np.float32)


# revision 23
# speedup vs baseline: 1.2319x; 1.0085x over previous
"""ANI-style per-species MLP (384->160->128->96->1, CELU) over [B=128, A=512]
atoms with species routing, atom-summed to [B]. 8-core SPMD Trainium2 kernel.

v2: atom-parallel sharding as before (species-grouped atoms dealt round-robin
to 8 cores, zero-AEV dummy padding corrected on host), with:
  - AEV streamed as fp8e4m3 (rhs) against bf16 weights (lhsT): 4x less DMA.
  - bf16 activations in SBUF; f32 PSUM accumulate.
  - celu split: V1 = exp(ACT) + clamp(DVE ts, bf16 4x mode) + stt(DVE);
    V2 (layer1) = exp+relu on ACT + clamp + tensor_add(DVE 2x) to balance
    ACT vs DVE occupancy.
  - L0's 32-wide output chunk for the 4 tiles of a batch lands in ONE psum
    bank at partition offsets 32j -> one merged celu instead of 4.
  - software-pipelined emission B(k) -> A(k+1) -> C(k) so the PE queue never
    waits on celu latency; PSUM pools shared (4 + 3 + 1 banks).
"""

import os
import sys

import numpy as np

try:
    import concourse  # noqa: F401
except ImportError:
    sys.path.insert(0, "/opt/trn_rl_repo")

N_CORES = 8
B, A, FEAT = 128, 512, 384
N_SPECIES = 4
H0, H1, H2 = 160, 128, 96
ALPHA = 0.1
LNA = float(np.log(ALPHA))

WPS = 3 * 160 + 128 + 96 + 1  # 705 weight-pack columns per species
CPS = 6                       # constant-pack columns per species

X_FP8 = bool(int(os.environ.get("BASSNN_X_FP8", "1")))
L0_DP = bool(int(os.environ.get("BASSNN_L0_DP", "1")))

TRACE = bool(int(os.environ.get("BASSNN_TRACE", "0")))
LAST = {}

_progs = {}


def _maybe_register_ntff_hook():
    try:
        import types

        import antenv
        from antenv import axon_hooks  # noqa: F401
        return
    except ImportError:
        pass
    try:
        import types

        import antenv
        from trn_agent_boot.trn_boot import _ntff_profile_via_ctypes

        mod = types.ModuleType("antenv.axon_hooks")
        holder = [None]
        mod.set_axon_ntff_profile_hook = lambda h: holder.__setitem__(0, h)
        mod.get_axon_ntff_profile_hook = lambda: holder[0]
        sys.modules["antenv.axon_hooks"] = mod
        antenv.axon_hooks = mod
        mod.set_axon_ntff_profile_hook(
            _ntff_profile_via_ctypes("/opt/axon/libaxon_pjrt.so")
        )
    except Exception:
        pass


def _tiles_for_groups(G):
    """Per-species padded group sizes -> list of (species, slot0, n_atoms)."""
    tiles = []
    slot0 = 0
    for s, g in enumerate(G):
        a = 0
        while a < g:
            na = 4 if g - a >= 4 else g - a
            tiles.append((s, slot0 + a, na))
            a += na
        slot0 += g
    return tiles


def _batches_for_tiles(tiles):
    return [list(range(b, min(b + 4, len(tiles)))) for b in range(0, len(tiles), 4)]


def _build_program(G, S):
    import concourse.bass as bass  # noqa: F401
    import concourse.tile as tile
    from concourse import bacc, mybir

    F32 = mybir.dt.float32
    BF16 = mybir.dt.bfloat16
    X8 = mybir.dt.float8e4 if X_FP8 else BF16
    EXP = mybir.ActivationFunctionType.Exp
    RELU = mybir.ActivationFunctionType.Relu
    MIN = mybir.AluOpType.min
    MAX = mybir.AluOpType.max
    ADD = mybir.AluOpType.add
    SUB = mybir.AluOpType.subtract

    tiles = _tiles_for_groups(G)
    ntiles = len(tiles)
    batches = _batches_for_tiles(tiles)
    nbatches = len(batches)
    WB0 = WPS * N_SPECIES
    CB0 = CPS * N_SPECIES
    WCOLS = WB0 + 128 * nbatches
    CCOLS = CB0 + 2 * nbatches
    # batch k covers consecutive slots [bslot0[k], bslot0[k] + bna[k])
    bslot0 = [tiles[bt[0]][1] for bt in batches]
    bna = [sum(tiles[ti][2] for ti in bt) for bt in batches]
    XB = max(bna) * 384

    nc = bacc.Bacc("TRN2", target_bir_lowering=False, debug=False,
                   num_devices=N_CORES)
    xt = nc.dram_tensor("xt", [128, S * 384], X8, kind="ExternalInput").ap()
    wp = nc.dram_tensor("wp", [128, WCOLS], BF16, kind="ExternalInput").ap()
    cp = nc.dram_tensor("cp", [128, CCOLS], F32, kind="ExternalInput").ap()
    yo = nc.dram_tensor("yo", [1, 512], F32, kind="ExternalOutput").ap()

    with tile.TileContext(nc) as tc:
        with (
            tc.tile_pool(name="wpool", bufs=1) as wpool,
            tc.tile_pool(name="cpool", bufs=1) as cpool,
            tc.tile_pool(name="xpool", bufs=3) as xpool,
            tc.tile_pool(name="epool", bufs=6) as epool,
            tc.tile_pool(name="tpool", bufs=6) as tpool,
            tc.tile_pool(name="rpool", bufs=3) as rpool,
            tc.tile_pool(name="y0apool", bufs=6) as y0apool,
            tc.tile_pool(name="y0bpool", bufs=2) as y0bpool,
            tc.tile_pool(name="y1pool", bufs=5) as y1pool,
            tc.tile_pool(name="y2pool", bufs=6) as y2pool,
            tc.tile_pool(name="opool", bufs=1) as opool,
            tc.tile_pool(name="ppa", bufs=2, space="PSUM") as ppa,
            tc.tile_pool(name="pp1", bufs=2, space="PSUM") as pp1,
            tc.tile_pool(name="ppb0", bufs=1, space="PSUM") as ppb0,
            tc.tile_pool(name="ppb2", bufs=2, space="PSUM") as ppb2,
            tc.tile_pool(name="pp3", bufs=1, space="PSUM") as pp3,
        ):
            # weights/constants on the gpsimd DMA queue, x batches on sync:
            # descriptor generation and transfers overlap. The first species'
            # weight block and the constants go first so the opening matmuls
            # and celus are not gated on the full weight pack (subtile deps).
            w = wpool.tile([128, WCOLS], BF16)
            c = cpool.tile([128, CCOLS], F32)
            nc.gpsimd.dma_start(c[:], cp[:])
            s_first = tiles[0][0]
            wA, wB = WPS * s_first, WPS * s_first + WPS
            nc.gpsimd.dma_start(w[:, wA:wB], wp[:, wA:wB])
            if wA > 0:
                nc.gpsimd.dma_start(w[:, 0:wA], wp[:, 0:wA])
            nc.gpsimd.dma_start(w[:, wB:WCOLS], wp[:, wB:WCOLS])

            def wcol(s, off, n):
                return w[:, s * WPS + off: s * WPS + off + n]

            def ccol(s, k, parts):
                return c[0:parts, s * CPS + k: s * CPS + k + 1]

            p3 = pp3.tile([1, 512], F32)

            def celu_v1(y_ap, p_ap, e_ap, t_ap, ebias, mbias):
                # y = (P max mbias) + min(alpha*e^(10P+ebias) - alpha, 0)
                nc.scalar.activation(e_ap, p_ap, EXP, bias=ebias, scale=10.0)
                nc.vector.tensor_scalar(t_ap, e_ap, ALPHA, 0.0, SUB, MIN)
                nc.vector.scalar_tensor_tensor(y_ap, p_ap, mbias, t_ap,
                                               MAX, ADD)

            def celu_v2(y_ap, p_ap, e_ap, t_ap, r_ap, ebias, rbias):
                # y = relu(P + rbias) + min(alpha*e^(10P+ebias) - alpha, 0)
                nc.scalar.activation(e_ap, p_ap, EXP, bias=ebias, scale=10.0)
                nc.scalar.activation(r_ap, p_ap, RELU, bias=rbias, scale=1.0)
                nc.vector.tensor_scalar(t_ap, e_ap, ALPHA, 0.0, SUB, MIN)
                nc.vector.tensor_add(y_ap, r_ap, t_ap)

            xts = {}

            def dma_batch(k, split_first=False):
                t = xpool.tile([128, XB], X8, name="xts")
                o = bslot0[k] * 384
                if split_first:
                    n0 = tiles[batches[k][0]][2] * 384
                    nc.sync.dma_start(t[:, 0:n0], xt[:, o: o + n0])
                    nc.sync.dma_start(t[:, n0: bna[k] * 384],
                                      xt[:, o + n0: o + bna[k] * 384])
                else:
                    nc.sync.dma_start(t[:, 0: bna[k] * 384],
                                      xt[:, o: o + bna[k] * 384])
                xts[k] = t

            y0as = {}
            y0bs = {}
            y1s = {}
            y2s = {}
            p0bs = {}

            def l0_mm_thunks(k):
                """Batch k's L0 matmuls as 12 thunks (2 mms each)."""
                batch = batches[k]
                xv = xts[k].rearrange("p (a f m) -> p a f m",
                                      a=XB // 384, f=3, m=128)
                p0b = ppb0.tile([128, 512], F32, tag="pb", name="p0b")
                p0bs[k] = p0b
                p0as = {}
                thunks = []
                for j, ti in enumerate(batch):
                    s, a0, na = tiles[ti]
                    N = na * 128
                    aoff = a0 - bslot0[k]
                    p0a = ppa.tile([128, 512], F32, tag="pa", name="p0a")
                    p0as[ti] = p0a

                    pm = (mybir.MatmulPerfMode.DoublePixel
                          if (L0_DP and X_FP8) else None)

                    def mk(j=j, s=s, na=na, N=N, aoff=aoff, p0a=p0a,
                           pm=pm, fc=0):
                        rhs = xv[:, aoff: aoff + na, fc, :]
                        nc.tensor.matmul(p0a[:, 0:N],
                                         wcol(s, fc * 160, 128),
                                         rhs, start=(fc == 0), stop=(fc == 2),
                                         perf_mode=pm)
                        nc.tensor.matmul(p0b[32 * j: 32 * j + 32, 0:N],
                                         wcol(s, fc * 160 + 128, 32),
                                         rhs, start=(fc == 0), stop=(fc == 2),
                                         tile_position=(0, 32 * j),
                                         perf_mode=pm)
                    for fc in range(3):
                        thunks.append(
                            lambda mk=mk, fc=fc: mk(fc=fc))
                return thunks, p0as

            def l0a_celu(k, j, p0as):
                ti = batches[k][j]
                s, a0, na = tiles[ti]
                N = na * 128
                ea = epool.tile([128, 512], BF16, name="ea")
                ta = tpool.tile([128, 512], BF16, name="ta")
                y0a = y0apool.tile([128, 512], BF16, name="y0a")
                celu_v1(y0a[:, 0:N], p0as[ti][:, 0:N], ea[:, 0:N],
                        ta[:, 0:N], ccol(s, 0, 128), ccol(s, 1, 128))
                y0as[ti] = y0a

            def l0b_celu(k):
                L = len(batches[k])
                em = epool.tile([128, 512], BF16, name="em")
                tm = tpool.tile([128, 512], BF16, name="tm")
                y0b = y0bpool.tile([128, 512], BF16, name="y0b")
                P = 32 * L
                celu_v1(y0b[0:P, :], p0bs[k][0:P, :], em[0:P, :], tm[0:P, :],
                        c[0:P, CB0 + 2 * k: CB0 + 2 * k + 1],
                        c[0:P, CB0 + 2 * k + 1: CB0 + 2 * k + 2])
                y0bs[k] = y0b

            def steady(k):
                """B(k) with A(k+1)'s L0 matmuls interleaved, then C(k)."""
                batch = batches[k]
                y0b = y0bs[k]
                if k + 2 < nbatches:
                    dma_batch(k + 2)
                p1s = {}
                for j, ti in enumerate(batch):
                    s, a0, na = tiles[ti]
                    N = na * 128
                    p1 = pp1.tile([128, 512], F32, tag="p1", name="p1")
                    nc.tensor.matmul(p1[:, 0:N], wcol(s, 480, 128),
                                     y0as[ti][:, 0:N], start=True, stop=False)
                    nc.tensor.matmul(
                        p1[:, 0:N],
                        w[32 * j: 32 * j + 32,
                          WB0 + 128 * k: WB0 + 128 * (k + 1)],
                        y0b[32 * j: 32 * j + 32, 0:N],
                        start=False, stop=True,
                        tile_position=(32 * j, 0))
                    p1s[ti] = p1
                for j, ti in enumerate(batch):
                    s, a0, na = tiles[ti]
                    N = na * 128
                    e1 = epool.tile([128, 512], BF16, name="e1")
                    t1 = tpool.tile([128, 512], BF16, name="t1")
                    r1 = rpool.tile([128, 512], BF16, name="r1")
                    y1 = y1pool.tile([128, 512], BF16, name="y1")
                    celu_v2(y1[:, 0:N], p1s[ti][:, 0:N], e1[:, 0:N],
                            t1[:, 0:N], r1[:, 0:N],
                            ccol(s, 2, 128), ccol(s, 3, 128))
                    y1s[ti] = y1
                if k + 1 < nbatches:
                    thunks, p0as_next = l0_mm_thunks(k + 1)
                else:
                    thunks, p0as_next = [], None
                chunks = [thunks[3 * i: 3 * i + 3] for i in range(4)]
                nnext = len(batches[k + 1]) if p0as_next is not None else 0
                for j, ti in enumerate(batch):
                    for th in chunks[j]:
                        th()
                    if p0as_next is not None and j < nnext:
                        l0a_celu(k + 1, j, p0as_next)
                    if j == len(batch) - 1:
                        # all of next batch's L0 matmuls are in by now:
                        # finish its celus so the merged-L0b stt lands well
                        # before the next cycle's L1b matmuls need it.
                        for ch in chunks[len(batch):]:
                            for th in ch:
                                th()
                        if p0as_next is not None:
                            for jj in range(len(batch), nnext):
                                l0a_celu(k + 1, jj, p0as_next)
                            l0b_celu(k + 1)
                    s, a0, na = tiles[ti]
                    N = na * 128
                    p2 = ppb2.tile([96, 512], F32, tag="p2", name="p2")
                    nc.tensor.matmul(p2[:, 0:N], wcol(s, 608, 96),
                                     y1s[ti][:, 0:N], start=True, stop=True)
                    e2 = epool.tile([96, 512], BF16, name="e2")
                    t2 = tpool.tile([96, 512], BF16, name="t2")
                    y2 = y2pool.tile([96, 512], BF16, name="y2")
                    celu_v1(y2[:, 0:N], p2[:, 0:N], e2[:, 0:N], t2[:, 0:N],
                            ccol(s, 4, 96), ccol(s, 5, 96))
                    y2s[ti] = y2
                for j, ti in enumerate(batch):
                    s, a0, na = tiles[ti]
                    N = na * 128
                    nc.tensor.matmul(p3[0:1, 0:N],
                                     wcol(s, 704, 1)[0:96, :],
                                     y2s[ti][0:96, 0:N],
                                     start=(ti == 0), stop=(ti == ntiles - 1),
                                     skip_group_check=True)

            dma_batch(0, split_first=True)
            if nbatches > 1:
                dma_batch(1)
            thunks0, p0as0 = l0_mm_thunks(0)
            for th in thunks0:
                th()
            for j in range(len(batches[0])):
                l0a_celu(0, j, p0as0)
            l0b_celu(0)
            for k in range(nbatches):
                steady(k)

            t3 = opool.tile([1, 512], F32)
            nc.scalar.copy(t3[:], p3[:])
            nc.sync.dma_start(yo[:], t3[:])

    nc.compile()
    return nc


def _celu64(z):
    return np.where(z > 0, z, ALPHA * np.expm1(np.minimum(z, 0) / ALPHA))


def _bf16_round(x):
    import ml_dtypes
    return np.asarray(x, np.float32).astype(ml_dtypes.bfloat16).astype(np.float64)


def kernel(fullaev, species, W0, b0, W1, b1, W2, b2, W3, b3):
    import ml_dtypes
    from concourse import bass_utils, mybir

    fullaev = np.ascontiguousarray(np.asarray(fullaev, dtype=np.float32))
    species = np.asarray(species, dtype=np.int32)
    Ws = [np.asarray(w, dtype=np.float32) for w in (W0, W1, W2, W3)]
    bs = [np.asarray(b, dtype=np.float32) for b in (b0, b1, b2, b3)]

    # --- species grouping: per-core slot assignment ---------------------
    ids = [np.where(species == s)[0] for s in range(N_SPECIES)]
    n = [len(i) for i in ids]
    G = [-(-n[s] // N_CORES) if n[s] else 0 for s in range(N_SPECIES)]
    S = sum(G)
    key = (tuple(G), X_FP8, L0_DP)
    if key not in _progs:
        _progs[key] = _build_program(G, S)
    nc = _progs[key]

    tiles = _tiles_for_groups(G)
    batches = _batches_for_tiles(tiles)
    nbatches = len(batches)
    WB0 = WPS * N_SPECIES
    CB0 = CPS * N_SPECIES
    WCOLS = WB0 + 128 * nbatches
    CCOLS = CB0 + 2 * nbatches

    # --- fold constants (float64, with bf16-rounded weights) ------------
    cpack = np.zeros((128, CCOLS), np.float32)
    wpack = np.zeros((128, WCOLS), np.float32)
    c3 = np.zeros(N_SPECIES)
    K0 = np.zeros(N_SPECIES)
    c1s = {}
    for s in range(N_SPECIES):
        w1, w2, w3 = (_bf16_round(Ws[l][s]) for l in (1, 2, 3))
        bb0, bb1, bb2, bb3 = (b[s].astype(np.float64) for b in bs)
        c1 = bb1 + w1 @ bb0
        c1s[s] = c1
        c3[s] = bb3[0] + w3[0] @ bb2
        # device contribution of a dummy (zero-AEV) atom, excluding c3
        y0d = _celu64(bb0) - bb0
        y1d = _celu64(w1 @ y0d + c1)
        y2d = _celu64(w2 @ y1d + bb2) - bb2
        K0[s] = w3[0] @ y2d

        cb = s * CPS
        cpack[:, cb + 0] = 10.0 * bb0[:128] + LNA
        cpack[:, cb + 1] = -bb0[:128]
        cpack[:, cb + 2] = 10.0 * c1 + LNA
        cpack[:, cb + 3] = c1
        cpack[:96, cb + 4] = 10.0 * bb2 + LNA
        cpack[:96, cb + 5] = -bb2

        wb = s * WPS
        for fc in range(3):
            blk = Ws[0][s][:, fc * 128:(fc + 1) * 128].T  # [128in, 160out]
            wpack[:, wb + fc * 160: wb + fc * 160 + 160] = blk
        wpack[:, wb + 480: wb + 608] = Ws[1][s][:, :128].T
        wpack[:, wb + 608: wb + 704] = Ws[2][s].T
        wpack[:96, wb + 704] = Ws[3][s][0, :]

    for bi, batch in enumerate(batches):
        for j, ti in enumerate(batch):
            s = tiles[ti][0]
            b0b = bs[0][s].astype(np.float64)[128:]
            cpack[32 * j: 32 * j + 32, CB0 + 2 * bi] = 10.0 * b0b + LNA
            cpack[32 * j: 32 * j + 32, CB0 + 2 * bi + 1] = -b0b
            wpack[32 * j: 32 * j + 32,
                  WB0 + 128 * bi: WB0 + 128 * (bi + 1)] = Ws[1][s][:, 128:].T

    wpack_b = wpack.astype(ml_dtypes.bfloat16)
    x_np_dtype = mybir.dt.np(mybir.dt.float8e4 if X_FP8 else mybir.dt.bfloat16)

    # --- per-core transposed, species-sorted AEV blocks -----------------
    in_maps = []
    dummy_counts = np.zeros((N_CORES, N_SPECIES), np.int64)
    for cid in range(N_CORES):
        xtc = np.zeros((128, S, 3, 128), np.float32)
        slot0 = 0
        for s in range(N_SPECIES):
            mine = ids[s][cid::N_CORES]
            nr = len(mine)
            dummy_counts[cid, s] = G[s] - nr
            if nr:
                g = fullaev[:, mine, :]               # [128, nr, 384]
                t = g.transpose(2, 1, 0)              # [384, nr, 128]
                xtc[:, slot0: slot0 + nr, :, :] = (
                    t.reshape(3, 128, nr, 128).transpose(1, 2, 0, 3)
                )
            slot0 += G[s]
        xq = xtc.reshape(128, S * 384).astype(x_np_dtype)
        in_maps.append({"xt": xq, "wp": wpack_b, "cp": cpack})

    if TRACE:
        _maybe_register_ntff_hook()
    res = bass_utils.run_bass_kernel_spmd(
        nc, in_maps, core_ids=list(range(N_CORES)), trace=TRACE
    )
    LAST["exec_time_ns"] = res.exec_time_ns
    LAST["trace"] = res.instructions_and_trace[1] if res.instructions_and_trace else None

    out = np.zeros(128, np.float64)
    for cid in range(N_CORES):
        out += (res.results[cid]["yo"][0].astype(np.float64)
                .reshape(4, 128).sum(axis=0))
    for s in range(N_SPECIES):
        out += n[s] * c3[s] - dummy_counts[:, s].sum() * K0[s]
    return out.astype(np.float32)
